# revision 1
# baseline (speedup 1.0000x reference)
"""DynamicSoftKMeansLoss on 8 Trainium2 NeuronCores.

Strategy (data-parallel over B, hardcoded for B=200000, D=256, K=5, C=16):
  - Host pads B to 8*25088 rows (pad labels=C so their one-hot is all-zero:
    padded rows contribute to no segment sum), shards rows across 8 cores and
    pre-transposes each shard to partition-major [128, T, 256] so every DMA
    descriptor is a contiguous >=1KB run.
  - Per 128-row tile on device: PE-transpose X, matmul against ctil = -2*C^T
    to get -2*x.c in PSUM, dist = sqrt(psum + |c|^2 + |x|^2); then softmax
    weighted dist wd, min / second-min over the 5 centers to get, for every
    hypothetical "closest center" j, viol_j = relu(wd + margin - min_{k!=j} d_k).
  - All per-class reductions are ONE accumulating matmul per tile:
    out[13, 16] += vals[r, 13]^T @ onehot(labels)[r, 16] with
    vals = [w*dist(5) | w*viol_j(5) | w*wd^2 | w | 1].
  - Tiny [13,16] AllReduce across the 8 cores, then on-device final stage:
    per-class argmin -> select viol sum -> per-class loss -> scalar.
"""

import sys

sys.path.insert(0, "/opt/trn_rl_repo")

import numpy as np

import concourse.bass as bass
import concourse.bacc as bacc
import concourse.tile as tile
from concourse import mybir
from concourse.bass_utils import run_bass_kernel_spmd

F32 = mybir.dt.float32
BF16 = mybir.dt.bfloat16
ALU = mybir.AluOpType
ACTF = mybir.ActivationFunctionType
AX = mybir.AxisListType

B, D, K, C = 200000, 256, 5, 16
NCORES = 8
MARGIN = 0.5
BIG = float(2.0**40)
BIGR = float(2.0**-40)

# Per-core geometry: T tiles of 128 rows.
TILES = 196          # 196*128 = 25088 rows/core; 8*25088 = 200704 >= 200000
RPC = TILES * 128
GB = 14              # tiles per G-batch (196 = 14*14)
NM = 13              # vals metrics: w*dist(5) | w*viol(5) | w*wd2 | w | 1
PRECISION = "bf16"   # which build kernel() uses


def _b0(ap, n, axis="inner"):
    """Stride-0 broadcast of a 2D [128, G] (or [128, C]) AP to 3D."""
    pairs = [list(p) for p in ap.ap]
    if axis == "inner":
        newap = pairs + [[0, n]]
    else:  # outer: [128, C] -> [128, n, C]
        newap = [pairs[0], [0, n], pairs[1]]
    return bass.AP(tensor=ap.tensor, offset=ap.offset, ap=newap)


def _patch_act_tables():
    """Placement-only hint: hide Ln/Exp from every table except the combined
    natural_log_exp_and_others so Bacc's greedy table-load placement picks the
    one table that serves Ln, Exp and Relu together (ids stay valid)."""
    import concourse.bacc as _bacc
    from concourse.hw_specs import get_activation_tables as _orig

    def patched(arch):
        tabs = _orig(arch)
        keep = "natural_log_exp_and_others"
        if keep in tabs:
            for name, funcs in tabs.items():
                if name != keep:
                    funcs.discard(ACTF.Ln)
                    funcs.discard(ACTF.Exp)
        return tabs

    _bacc.get_activation_tables = patched


def build_nc(tiles=TILES, gb=GB, n_cores=NCORES, precision="f32", repeat=1):
    if precision == "bf16":
        _patch_act_tables()
    nc = bacc.Bacc(None, num_devices=n_cores)
    nb = tiles // gb
    assert tiles % gb == 0

    if precision == "bf16":
        # host-pretransposed XT layout: [dpart, dchunk, tile, row]
        x_dram = nc.declare_dram_parameter(
            "x", [128, 2, tiles, 128], F32, isOutput=False
        )
    else:
        x_dram = nc.declare_dram_parameter("x", [128, tiles, D], F32, isOutput=False)
    # one packed constant tensor: ctil | cnorm | iota | eye | lab | w
    NCST = 2 * K + gb * K + C + 128 + 2 * tiles
    const_dram = nc.declare_dram_parameter("const", [128, NCST], F32, isOutput=False)
    cbf_dram = nc.declare_dram_parameter("cbf", [128, 3 * K], BF16, isOutput=False)
    out_dram = nc.declare_dram_parameter("out", [1, 1], F32, isOutput=True)

    cc_in = nc.dram_tensor("cc_in", [NM, C], F32)
    cc_out = nc.dram_tensor(
        "cc_out", [NM, C], F32, addr_space="Shared" if n_cores > 4 else "Local"
    )

    with tile.TileContext(nc) as tc:
        with (
            tc.tile_pool(name="consts", bufs=1) as consts,
            tc.tile_pool(name="xin", bufs=2) as xin,
            tc.tile_pool(name="xt", bufs=3) as xtp,
            tc.tile_pool(name="small", bufs=2) as small,
            tc.tile_pool(name="stat", bufs=2) as stat,
            tc.tile_pool(name="ps_xt", bufs=3, space="PSUM") as psxt,
            tc.tile_pool(name="ps_d", bufs=2, space="PSUM") as psd_pool,
            tc.tile_pool(name="ps_seg", bufs=1, space="PSUM") as psseg,
        ):
            const_sb = consts.tile([128, NCST], F32)
            nc.scalar.dma_start(const_sb[:], const_dram[:])
            cbf_sb = consts.tile([128, 3 * K], BF16)
            if precision == "bf16":
                nc.scalar.dma_start(cbf_sb[:], cbf_dram[:])
            o = 0
            ctil_sb = const_sb[:, o:o + 2 * K]; o += 2 * K
            cnorm_sb = const_sb[:, o:o + gb * K]; o += gb * K
            iota_sb = const_sb[:, o:o + C]; o += C
            eye_sb = const_sb[:, o:o + 128]; o += 128
            lab_sb = const_sb[:, o:o + tiles]; o += tiles
            w_sb = const_sb[:, o:o + tiles]; o += tiles

            psum_seg = psseg.tile([NM, C], F32)

            for rep in range(repeat):
              for b in range(nb):
                  if precision != "bf16":
                      norm2 = stat.tile([128, gb], F32)
                  psd = psd_pool.tile([128, gb, K], F32)
                  oh = small.tile([128, gb, C], F32, tag="oh")
                  vals = small.tile([128, gb, NM], F32, tag="vals")

                  if precision == "bf16":
                      # casting DMA (f32 DRAM -> bf16 SBUF) of pretransposed XT
                      xb = xin.tile([128, 2, gb, 128], BF16)
                      nc.gpsimd.dma_start(
                          xb[:], x_dram[:, :, b * gb:(b + 1) * gb, :]
                      )
                      # sq = xt*xt for the whole batch (one DVE op, bf16 2x)
                      sq = xtp.tile([128, 2, gb, 128], BF16)
                      nc.vector.tensor_tensor(sq[:], xb[:], xb[:], ALU.mult)
                      # one-hot for the whole batch via stride-0 broadcasts
                      lab_g = lab_sb[:, b * gb:(b + 1) * gb]
                      nc.vector.tensor_tensor(
                          oh[:], _b0(iota_sb, gb, "outer"),
                          _b0(lab_g, C, "inner"), ALU.is_equal,
                      )
                      for g in range(gb):
                          # psd[r,k] = -2 x.c  +  sum_d x^2  (ones columns)
                          nc.tensor.matmul(
                              psd[:, g, :], xb[:, 0, g, :], cbf_sb[:, 0:K],
                              start=True, stop=False,
                          )
                          nc.tensor.matmul(
                              psd[:, g, :], xb[:, 1, g, :], cbf_sb[:, K:2 * K],
                              start=False, stop=False,
                          )
                          nc.tensor.matmul(
                              psd[:, g, :], sq[:, 0, g, :], cbf_sb[:, 2 * K:3 * K],
                              start=False, stop=False,
                          )
                          nc.tensor.matmul(
                              psd[:, g, :], sq[:, 1, g, :], cbf_sb[:, 2 * K:3 * K],
                              start=False, stop=True,
                          )
                  else:
                      xb = xin.tile([128, gb, D], F32)
                      nc.sync.dma_start(xb[:], x_dram[:, b * gb:(b + 1) * gb, :])
                      for g in range(gb):
                          t = b * gb + g
                          xt_ps = psxt.tile([128, D], F32)
                          xt_sb = xtp.tile([128, D], F32)
                          nc.tensor.transpose(xt_ps[:, 0:128], xb[:, g, 0:128], eye_sb)
                          nc.tensor.transpose(xt_ps[:, 128:256], xb[:, g, 128:256], eye_sb)
                          nc.vector.tensor_copy(xt_sb[:], xt_ps[:])
                          nc.tensor.matmul(
                              psd[:, g, :], xt_sb[:, 0:128], ctil_sb[:, 0:K],
                              start=True, stop=False,
                          )
                          nc.tensor.matmul(
                              psd[:, g, :], xt_sb[:, 128:256], ctil_sb[:, K:2 * K],
                              start=False, stop=True,
                          )
                          # |x|^2 per row (free-dim accumulate on ACT)
                          dump = stat.tile([128, D], F32, tag="dump")
                          nc.scalar.activation(
                              dump[:], xb[:, g, :], ACTF.Square,
                              accum_out=norm2[:, g:g + 1],
                          )
                          nc.vector.tensor_scalar(
                              oh[:, g, :], iota_sb, lab_sb[:, t:t + 1], None,
                              ALU.is_equal,
                          )

                  # d2 = psum + |c|^2  (cnorm_sb is host-replicated per-tile block)
                  t_d2 = small.tile([128, gb, K], F32, tag="t_d2")
                  nc.vector.tensor_tensor(
                      t_d2[:], psd[:], cnorm_sb.rearrange("p (g k) -> p g k", k=K),
                      ALU.add,
                  )
                  # dist = sqrt(d2): bf16 path uses exp(0.5*ln(d2)) so Ln/Exp
                  # stay in one activation table; f32 path uses Sqrt + bias.
                  if precision == "bf16":
                      lnt = small.tile([128, gb, K], F32, tag="lnt")
                      nc.scalar.activation(lnt[:], t_d2[:], ACTF.Ln)
                      nc.scalar.activation(
                          vals[:, :, 0:K], lnt[:], ACTF.Exp, scale=0.5
                      )
                  else:
                      for g in range(gb):
                          nc.scalar.activation(
                              vals[:, g, 0:K], t_d2[:, g, :], ACTF.Sqrt,
                              bias=norm2[:, g:g + 1],
                          )
                  dist = vals[:, :, 0:K]

                  m1 = stat.tile([128, gb], F32, tag="m1")
                  nc.vector.tensor_reduce(m1[:], dist, axis=AX.X, op=ALU.min)
                  maskB = small.tile([128, gb, K], F32, tag="maskB")
                  eu = small.tile([128, gb, K], F32, tag="eu")
                  s = stat.tile([128, gb], F32, tag="s")
                  mo = small.tile([128, gb, K], F32, tag="mo")
                  dmask = small.tile([128, gb, K], F32, tag="dmask")
                  m2 = stat.tile([128, gb], F32, tag="m2")
                  deltaS = stat.tile([128, gb], F32, tag="deltaS")
                  if precision == "bf16":
                      # all broadcasts amortized over the whole batch
                      nc.vector.tensor_tensor(
                          maskB[:], dist, _b0(m1[:], K), ALU.is_equal
                      )
                      nc.gpsimd.tensor_scalar(
                          maskB[:], maskB[:], BIG, None, ALU.mult
                      )
                      nc.gpsimd.tensor_tensor(dmask[:], dist, maskB[:], ALU.add)
                      nc.vector.tensor_reduce(
                          m2[:], dmask[:], axis=AX.X, op=ALU.min
                      )
                      nc.vector.tensor_tensor(
                          deltaS[:], m2[:], m1[:], ALU.subtract
                      )
                      nc.vector.tensor_scalar(
                          deltaS[:], deltaS[:], BIGR, None, ALU.mult
                      )
                      nc.vector.tensor_tensor(
                          mo[:], maskB[:], _b0(deltaS[:], K), ALU.mult
                      )
                      nc.vector.tensor_tensor(
                          mo[:], mo[:], _b0(m1[:], K), ALU.add
                      )
                      # unnormalized softmax exp(-d) (values ~1e-6..1e-13, fine
                      # in f32; the max-subtraction cancels in the ratio)
                      nc.scalar.activation(eu[:], dist, ACTF.Exp, scale=-1.0)
                      nc.vector.tensor_reduce(s[:], eu[:], axis=AX.X, op=ALU.add)
                  else:
                      for g in range(gb):
                          nc.vector.tensor_scalar(
                              maskB[:, g, :], vals[:, g, 0:K], m1[:, g:g + 1],
                              BIG, ALU.is_equal, ALU.mult,
                          )
                      nc.gpsimd.tensor_tensor(dmask[:], dist, maskB[:], ALU.add)
                      nc.vector.tensor_reduce(
                          m2[:], dmask[:], axis=AX.X, op=ALU.min
                      )
                      nc.vector.tensor_tensor(
                          deltaS[:], m2[:], m1[:], ALU.subtract
                      )
                      nc.vector.tensor_scalar(
                          deltaS[:], deltaS[:], BIGR, None, ALU.mult
                      )
                      for g in range(gb):
                          nc.vector.tensor_scalar(
                              mo[:, g, :], maskB[:, g, :],
                              deltaS[:, g:g + 1], m1[:, g:g + 1],
                              ALU.mult, ALU.add,
                          )
                      for g in range(gb):
                          nc.scalar.activation(
                              eu[:, g, :], vals[:, g, 0:K], ACTF.Exp,
                              bias=m1[:, g:g + 1], scale=-1.0,
                          )
                      nc.vector.tensor_reduce(s[:], eu[:], axis=AX.X, op=ALU.add)
                  prod = small.tile([128, gb, K], F32, tag="prod")
                  nc.vector.tensor_tensor(prod[:], eu[:], dist, ALU.mult)
                  spd = stat.tile([128, gb], F32, tag="spd")
                  nc.vector.tensor_reduce(spd[:], prod[:], axis=AX.X, op=ALU.add)
                  rs = stat.tile([128, gb], F32, tag="rs")
                  nc.vector.reciprocal(rs[:], s[:])
                  wd = stat.tile([128, gb], F32, tag="wd")
                  nc.vector.tensor_tensor(wd[:], spd[:], rs[:], ALU.mult)
                  wdp = stat.tile([128, gb], F32, tag="wdp")
                  nc.vector.tensor_scalar(wdp[:], wd[:], MARGIN, None, ALU.add)
                  # vals[:, :, 10] = wd^2 ; vals[:, :, 11:13] = 1
                  wd3 = wd[:].rearrange("p (g o) -> p g o", o=1)
                  nc.vector.tensor_tensor(vals[:, :, 10:11], wd3, wd3, ALU.mult)
                  nc.gpsimd.memset(vals[:, :, 11:13], 1.0)
                  # vals[:, :, 5:10] = viol_j = relu(wdp - mo_j)
                  if precision == "bf16":
                      hng = small.tile([128, gb, K], F32, tag="hng")
                      nc.vector.tensor_tensor(
                          hng[:], mo[:], _b0(wdp[:], K), ALU.subtract
                      )
                      nc.scalar.activation(
                          vals[:, :, K:2 * K], hng[:], ACTF.Relu, scale=-1.0
                      )
                      # weight cols 0..11 by w in one broadcasted op
                      w_g = w_sb[:, b * gb:(b + 1) * gb]
                      nc.vector.tensor_tensor(
                          vals[:, :, 0:12], vals[:, :, 0:12],
                          _b0(w_g, 12), ALU.mult,
                      )
                  else:
                      for g in range(gb):
                          nc.scalar.activation(
                              vals[:, g, K:2 * K], mo[:, g, :], ACTF.Relu,
                              bias=wdp[:, g:g + 1], scale=-1.0,
                          )
                      for g in range(gb):
                          t = b * gb + g
                          nc.vector.tensor_scalar(
                              vals[:, g, 0:12], vals[:, g, 0:12],
                              w_sb[:, t:t + 1], None, ALU.mult,
                          )
                  # segment accumulate: psum_seg[13, 16] += vals^T @ onehot
                  for g in range(gb):
                      t = b * gb + g
                      nc.tensor.matmul(
                          psum_seg[:], vals[:, g, :], oh[:, g, :],
                          start=(rep == 0 and t == 0),
                        stop=(rep == repeat - 1 and t == tiles - 1),
                      )

            # ---- cross-core all-reduce of the [13, 16] stats ----
            seg_sb = consts.tile([NM, C], F32, tag="seg_sb")
            nc.vector.tensor_copy(seg_sb[:], psum_seg[:])
            nc.sync.dma_start(cc_in[:], seg_sb[:])
            if n_cores > 1:
                nc.gpsimd.collective_compute(
                    "AllReduce",
                    ALU.add,
                    replica_groups=[list(range(n_cores))],
                    ins=[cc_in.ap().opt()],
                    outs=[cc_out.ap().opt()],
                )
                red_src = cc_out
            else:
                red_src = cc_in
            segr = consts.tile([NM, C], F32, tag="segr")
            nc.sync.dma_start(segr[:], red_src[:])

            # ---- final stage (tiny) ----
            with tc.tile_pool(name="ps_fin", bufs=1, space="PSUM") as psfin:
                segT_ps = psfin.tile([C, NM], F32)
                nc.tensor.transpose(segT_ps[:], segr[:], eye_sb[0:NM, 0:NM])
                segT = consts.tile([C, NM], F32, tag="segT")
                nc.vector.tensor_copy(segT[:], segT_ps[:])

                safe = consts.tile([C, 1], F32, tag="safe")
                nc.vector.tensor_scalar(safe[:], segT[:, 11:12], 1.0, None, ALU.max)
                rsafe = consts.tile([C, 1], F32, tag="rsafe")
                nc.vector.reciprocal(rsafe[:], safe[:])
                meand = consts.tile([C, K], F32, tag="meand")
                nc.vector.tensor_scalar(
                    meand[:], segT[:, 0:K], rsafe[:], None, ALU.mult
                )
                mind = consts.tile([C, 1], F32, tag="mind")
                nc.vector.tensor_reduce(mind[:], meand[:], axis=AX.X, op=ALU.min)
                cmask = consts.tile([C, K], F32, tag="cmask")
                nc.vector.tensor_scalar(
                    cmask[:], meand[:], mind[:], None, ALU.is_equal
                )
                sv = consts.tile([C, K], F32, tag="sv")
                nc.vector.tensor_tensor(sv[:], cmask[:], segT[:, K:2 * K], ALU.mult)
                svs = consts.tile([C, 1], F32, tag="svs")
                nc.vector.tensor_reduce(svs[:], sv[:], axis=AX.X, op=ALU.add)
                # pc2 col0 = per_class, col1 = present?
                pc2 = consts.tile([C, 2], F32, tag="pc2")
                num = consts.tile([C, 1], F32, tag="num")
                nc.vector.tensor_tensor(num[:], segT[:, 10:11], svs[:], ALU.add)
                nc.vector.tensor_scalar(num[:], num[:], rsafe[:], None, ALU.mult)
                has = consts.tile([C, 1], F32, tag="has")
                nc.vector.tensor_scalar(has[:], segT[:, 11:12], 0.0, None, ALU.is_gt)
                nc.vector.tensor_tensor(pc2[:, 0:1], num[:], has[:], ALU.mult)
                nc.vector.tensor_scalar(
                    pc2[:, 1:2], segT[:, 12:13], 0.0, None, ALU.is_gt
                )
                # column sums over the 16 classes via PE: [1,16] ones^T @ pc2
                ones16 = consts.tile([C, 1], F32, tag="ones16")
                nc.vector.memset(ones16[:], 1.0)
                fin_ps = psfin.tile([1, 2], F32, tag="fin")
                nc.tensor.matmul(fin_ps[:], ones16[:], pc2[:], start=True, stop=True)
                fin = consts.tile([1, 2], F32, tag="fin_sb")
                nc.vector.tensor_copy(fin[:], fin_ps[:])
                nuq = consts.tile([1, 1], F32, tag="nuq")
                nc.vector.tensor_scalar(nuq[:], fin[:, 1:2], 1.0, None, ALU.max)
                rnuq = consts.tile([1, 1], F32, tag="rnuq")
                nc.vector.reciprocal(rnuq[:], nuq[:])
                loss = consts.tile([1, 1], F32, tag="loss")
                nc.vector.tensor_scalar(
                    loss[:], fin[:, 0:1], rnuq[:], None, ALU.mult
                )
                nc.sync.dma_start(out_dram[:], loss[:])

    nc.compile()
    return nc


def _host_prep(feat, labels, label2, centers, tiles=TILES, gb=GB, n_cores=NCORES,
               precision=PRECISION):
    """Pad + shard + pre-transpose to partition-major per-core arrays."""
    rpc = tiles * 128
    bpad = rpc * n_cores
    b = feat.shape[0]
    gb_eff = gb

    feat = np.asarray(feat, dtype=np.float32)
    labels = np.asarray(labels)
    label2 = np.asarray(label2)
    centers = np.asarray(centers, dtype=np.float32)

    lab_f = np.full(bpad, float(C), dtype=np.float32)
    lab_f[:b] = labels.astype(np.float32)
    w_f = np.zeros(bpad, dtype=np.float32)
    w_f[:b] = (label2 == 1).astype(np.float32)
    xpad = np.zeros((bpad, D), dtype=np.float32)
    xpad[:b] = feat

    # constants
    ctilT = (-2.0 * centers.T).astype(np.float32)          # [256, 5]
    ctil = np.concatenate([ctilT[0:128], ctilT[128:256]], axis=1)  # [128, 10]
    cnorm = (centers * centers).sum(axis=1).astype(np.float32)     # [5]
    cnorm_rep = np.tile(cnorm[None, None, :], (128, gb_eff, 1)).reshape(
        128, gb_eff * K
    )
    iota = np.tile(np.arange(C, dtype=np.float32)[None, :], (128, 1))
    eye = np.eye(128, dtype=np.float32)

    import ml_dtypes
    cbf = np.concatenate(
        [ctil, np.ones((128, K), np.float32)], axis=1
    ).astype(ml_dtypes.bfloat16)                                   # [128, 15]
    in_maps = []
    for i in range(n_cores):
        sl = slice(i * rpc, (i + 1) * rpc)
        if precision == "bf16":
            # XT layout [dpart, dchunk, tile, row]:
            #   x[dp, c, t, r] = feat[t*128 + r, c*128 + dp]
            xi = np.ascontiguousarray(
                xpad[sl].reshape(tiles, 128, 2, 128).transpose(3, 2, 0, 1)
            )
        else:
            xi = np.ascontiguousarray(
                xpad[sl].reshape(tiles, 128, D).transpose(1, 0, 2)
            )
        li = np.ascontiguousarray(lab_f[sl].reshape(tiles, 128).T)
        wi = np.ascontiguousarray(w_f[sl].reshape(tiles, 128).T)
        const = np.concatenate(
            [ctil, cnorm_rep.astype(np.float32), iota, eye, li, wi], axis=1
        )
        in_maps.append(
            {"x": xi, "const": np.ascontiguousarray(const), "cbf": cbf}
        )
    return in_maps


_NC_CACHE = {}


def kernel(feat_normed, labels, label2, num_classes, centers, _trace=False):
    key = PRECISION
    if key not in _NC_CACHE:
        _NC_CACHE[key] = build_nc(precision=PRECISION)
    nc = _NC_CACHE[key]
    in_maps = _host_prep(
        feat_normed, labels, label2, centers, precision=PRECISION
    )
    res = run_bass_kernel_spmd(
        nc, in_maps, core_ids=list(range(NCORES)), trace=_trace
    )
    out = np.float32(res.results[0]["out"][0, 0])
    if _trace:
        kernel.last_result = res
    return np.asarray(out, dtype=np.float32)



# revision 2
# speedup vs baseline: 3.3178x; 3.3178x over previous
"""DynamicSoftKMeansLoss on 8 Trainium2 NeuronCores.

Strategy (data-parallel over B, hardcoded for B=200000, D=256, K=5, C=16):
  - Host pads B to 8*25088 rows, shards across 8 cores, pre-transposes each
    shard to partition-major XT [128 dpart, 2 dchunk, T tiles, 128 rows] and
    casts to fp8 e4m3 (x scaled by 16, centers by 1/16 so the PE product is
    unscaled). Host also precomputes per-row |x|^2 (exact, f32), the combined
    d2add[r,t,k] = |x_rt|^2 + |c_k|^2 table, w-folded labels
    wlab = label if label2==1 else C (so the one-hot is w-weighted for free),
    and the per-class counts/presence (needed only by the final scalar math).
  - Per 49-tile G-batch on device: 2 fp8 matmuls per tile -> -2 x.c in PSUM;
    dist = exp(0.5*ln(psum + d2add)); softmax weighted dist wd; min/2nd-min
    over the 5 centers; viol_j = relu(wd + margin - min_{k!=j} d_k) built as
    v1 + mask*(v2-v1). All per-class reductions are packed accumulating
    matmuls: psum[77,112] += vals7^T @ oh7 where 7 tiles' [128,11] vals /
    [128,16] one-hots are fused into one PE op (11 metrics: dist(5) |
    viol(5) | wd^2, all w-weighted via the one-hot).
  - Each core DMAs its [77,112] partial to DRAM; host sums the 8 partials,
    extracts the 7 diagonal [11,16] blocks, and does the tiny per-class
    argmin + final reduction in numpy (replaces a ~55us on-device
    collective+final-stage tail).
"""

import sys

sys.path.insert(0, "/opt/trn_rl_repo")

import numpy as np

import concourse.bass as bass
import concourse.bacc as bacc
import concourse.tile as tile
from concourse import mybir
from concourse.bass_utils import run_bass_kernel_spmd

F32 = mybir.dt.float32
BF16 = mybir.dt.bfloat16
F8 = mybir.dt.float8e4
ALU = mybir.AluOpType
ACTF = mybir.ActivationFunctionType
AX = mybir.AxisListType

B, D, K, C = 200000, 256, 5, 16
NCORES = 8
MARGIN = 0.5
BIG = float(2.0**40)

TILES = 196          # 196*128 = 25088 rows/core; 8*25088 = 200704 >= 200000
RPC = TILES * 128
GB = 49              # tiles per G-batch (196 = 4*49)
NM = 11              # vals metrics: w*dist(5) | w*viol(5) | w*wd2
SEGP = 7             # tiles packed per segment matmul (49 = 7*7)
XSCALE = 16.0        # fp8 range centering: x*16 on host, centers/16


def _b0(ap, n, axis="inner"):
    """Stride-0 broadcast of a 2D [128, G] (or [128, C]) AP to 3D."""
    pairs = [list(p) for p in ap.ap]
    if axis == "inner":
        newap = pairs + [[0, n]]
    else:  # outer: [128, C] -> [128, n, C]
        newap = [pairs[0], [0, n], pairs[1]]
    return bass.AP(tensor=ap.tensor, offset=ap.offset, ap=newap)


def _patch_act_tables():
    """Placement-only hint: hide Ln/Exp from every table except the combined
    natural_log_exp_and_others so Bacc's greedy table-load placement picks the
    one table that serves Ln, Exp, Relu and Copy together (ids stay valid)."""
    import concourse.bacc as _bacc
    from concourse.hw_specs import get_activation_tables as _orig

    def patched(arch):
        tabs = _orig(arch)
        keep = "natural_log_exp_and_others"
        if keep in tabs:
            for name, funcs in tabs.items():
                if name != keep:
                    funcs.discard(ACTF.Ln)
                    funcs.discard(ACTF.Exp)
        return tabs

    _bacc.get_activation_tables = patched


def build_nc(tiles=TILES, gb=GB, n_cores=NCORES):
    _patch_act_tables()
    nc = bacc.Bacc(None, num_devices=n_cores)
    nb = tiles // gb
    assert tiles % gb == 0 and gb % SEGP == 0

    # host-pretransposed, fp8-cast XT layout: [dpart, dchunk, tile, row]
    x_dram = nc.declare_dram_parameter("x", [128, 2, tiles, 128], F8, isOutput=False)
    # f32 consts: d2add [128, tiles*K]
    const_dram = nc.declare_dram_parameter(
        "const", [128, tiles * K], F32, isOutput=False
    )
    # bf16 consts: iota [128, C] | wlab [128, tiles]
    cbf_dram = nc.declare_dram_parameter("cbf", [128, C + tiles], BF16, isOutput=False)
    # fp8 consts: ctil (two 128-chunks of -2*C^T/XSCALE^2... see host prep)
    cf8_dram = nc.declare_dram_parameter("cf8", [128, 2 * K], F8, isOutput=False)
    out_dram = nc.declare_dram_parameter(
        "out", [SEGP * NM, SEGP * C], F32, isOutput=True
    )

    with tile.TileContext(nc) as tc:
        with (
            tc.tile_pool(name="consts", bufs=1) as consts,
            tc.tile_pool(name="xin", bufs=3) as xin,
            tc.tile_pool(name="big", bufs=2) as big,
            tc.tile_pool(name="stat", bufs=2) as stat,
            tc.tile_pool(name="ps_d", bufs=2, space="PSUM") as psd_pool,
            tc.tile_pool(name="ps_seg", bufs=1, space="PSUM") as psseg,
        ):
            const_sb = consts.tile([128, tiles * K], F32)
            nc.scalar.dma_start(const_sb[:], const_dram[:])
            cbf_sb = consts.tile([128, C + tiles], BF16)
            nc.scalar.dma_start(cbf_sb[:], cbf_dram[:])
            cf8_sb = consts.tile([128, 2 * K], F8)
            nc.scalar.dma_start(cf8_sb[:], cf8_dram[:])

            iota_sb = cbf_sb[:, 0:C]
            wlab_sb = cbf_sb[:, C:C + tiles]

            psum_seg = psseg.tile([SEGP * NM, SEGP * C], F32)

            for b in range(nb):
                xb = xin.tile([128, 2, gb, 128], F8)
                nc.sync.dma_start(xb[:], x_dram[:, :, b * gb:(b + 1) * gb, :])

                psd = psd_pool.tile([128, gb, K], F32)
                for g in range(gb):
                    nc.tensor.matmul(
                        psd[:, g, :], xb[:, 0, g, :], cf8_sb[:, 0:K],
                        start=True, stop=False,
                    )
                    nc.tensor.matmul(
                        psd[:, g, :], xb[:, 1, g, :], cf8_sb[:, K:2 * K],
                        start=False, stop=True,
                    )

                # w-weighted one-hot for the whole batch via stride-0 bcasts
                oh = big.tile([128, gb, C], BF16, tag="oh")
                nc.vector.tensor_tensor(
                    oh[:], _b0(iota_sb, gb, "outer"),
                    _b0(wlab_sb[:, b * gb:(b + 1) * gb], C, "inner"),
                    ALU.is_equal,
                )

                # d2 = psum + (|x|^2 + |c|^2); dist = exp(0.5*ln(d2)) so Ln,
                # Exp, Relu, Copy all come from one activation table.
                t_d2 = big.tile([128, gb, K], F32, tag="t_d2")
                nc.vector.tensor_tensor(
                    t_d2[:], psd[:],
                    const_sb[:, b * gb * K:(b + 1) * gb * K].rearrange(
                        "p (g k) -> p g k", k=K
                    ),
                    ALU.add,
                )
                lnt = big.tile([128, gb, K], F32, tag="lnt")
                nc.scalar.activation(lnt[:], t_d2[:], ACTF.Ln)
                dist = big.tile([128, gb, K], F32, tag="dist")
                nc.scalar.activation(dist[:], lnt[:], ACTF.Exp, scale=0.5)

                vals = big.tile([128, gb, NM], BF16, tag="vals")
                nc.scalar.activation(vals[:, :, 0:K], dist[:], ACTF.Copy)

                # min and masked 2nd-min over the 5 centers
                m1 = stat.tile([128, gb], F32, tag="m1")
                nc.vector.tensor_reduce(m1[:], dist[:], axis=AX.X, op=ALU.min)
                mask = big.tile([128, gb, K], F32, tag="mask")
                nc.vector.tensor_tensor(
                    mask[:], dist[:], _b0(m1[:], K), ALU.is_equal
                )
                dmask = big.tile([128, gb, K], F32, tag="dmask")
                nc.vector.scalar_tensor_tensor(
                    dmask[:], mask[:], BIG, dist[:], ALU.mult, ALU.add
                )
                m2 = stat.tile([128, gb], F32, tag="m2")
                nc.vector.tensor_reduce(m2[:], dmask[:], axis=AX.X, op=ALU.min)

                # softmax-weighted distance wd (unnormalized exp is fine in
                # f32; the max-subtraction cancels in the ratio)
                eu = big.tile([128, gb, K], F32, tag="eu")
                nc.scalar.activation(eu[:], dist[:], ACTF.Exp, scale=-1.0)
                s = stat.tile([128, gb], F32, tag="s")
                nc.vector.tensor_reduce(s[:], eu[:], axis=AX.X, op=ALU.add)
                prod = big.tile([128, gb, K], F32, tag="prod")
                nc.vector.tensor_tensor(prod[:], eu[:], dist[:], ALU.mult)
                spd = stat.tile([128, gb], F32, tag="spd")
                nc.vector.tensor_reduce(spd[:], prod[:], axis=AX.X, op=ALU.add)
                rs = stat.tile([128, gb], F32, tag="rs")
                nc.vector.reciprocal(rs[:], s[:])
                wd = stat.tile([128, gb], F32, tag="wd")
                nc.vector.tensor_tensor(wd[:], spd[:], rs[:], ALU.mult)

                wd3 = wd[:].rearrange("p (g o) -> p g o", o=1)
                nc.vector.tensor_tensor(
                    vals[:, :, 10:11], wd3, wd3, ALU.mult
                )

                # viol_j = relu(wd+m - min_{k!=j} d_k) = v1 + mask*(v2-v1)
                t1 = stat.tile([128, gb], F32, tag="t1")
                nc.vector.scalar_tensor_tensor(
                    t1[:], wd[:], MARGIN, m1[:], ALU.add, ALU.subtract
                )
                t2 = stat.tile([128, gb], F32, tag="t2")
                nc.vector.scalar_tensor_tensor(
                    t2[:], wd[:], MARGIN, m2[:], ALU.add, ALU.subtract
                )
                v1 = stat.tile([128, gb], F32, tag="v1")
                nc.scalar.activation(v1[:], t1[:], ACTF.Relu)
                v2 = stat.tile([128, gb], F32, tag="v2")
                nc.scalar.activation(v2[:], t2[:], ACTF.Relu)
                dv = stat.tile([128, gb], F32, tag="dv")
                nc.vector.tensor_tensor(dv[:], v2[:], v1[:], ALU.subtract)
                vtmp = big.tile([128, gb, K], F32, tag="vtmp")
                nc.vector.tensor_tensor(
                    vtmp[:], mask[:], _b0(dv[:], K), ALU.mult
                )
                nc.vector.tensor_tensor(
                    vals[:, :, K:2 * K], vtmp[:], _b0(v1[:], K), ALU.add
                )

                # segment accumulate, 7 tiles per PE op:
                # psum[77, 112] += vals[:, 7p:7p+7, :]^T @ oh[:, 7p:7p+7, :]
                npk = gb // SEGP
                for p in range(npk):
                    t = b * npk + p
                    nc.tensor.matmul(
                        psum_seg[:],
                        vals[:, p * SEGP:(p + 1) * SEGP, :].rearrange(
                            "p g m -> p (g m)"
                        ),
                        oh[:, p * SEGP:(p + 1) * SEGP, :].rearrange(
                            "p g c -> p (g c)"
                        ),
                        start=(t == 0),
                        stop=(t == nb * npk - 1),
                    )

            seg_sb = consts.tile([SEGP * NM, SEGP * C], F32, tag="seg_sb")
            nc.vector.tensor_copy(seg_sb[:], psum_seg[:])
            nc.sync.dma_start(out_dram[:], seg_sb[:])

    nc.compile()
    return nc


def _host_prep(feat, labels, label2, centers, tiles=TILES, n_cores=NCORES):
    """Pad + shard + pre-transpose + fp8-cast to per-core input maps."""
    import ml_dtypes

    rpc = tiles * 128
    bpad = rpc * n_cores
    b = feat.shape[0]

    feat = np.asarray(feat, dtype=np.float32)
    labels = np.asarray(labels)
    label2 = np.asarray(label2)
    centers = np.asarray(centers, dtype=np.float32)

    xpad = np.zeros((bpad, D), dtype=np.float32)
    xpad[:b] = feat
    norm2 = np.einsum("ij,ij->i", xpad, xpad, dtype=np.float32)
    x_f8 = (xpad * XSCALE).astype(ml_dtypes.float8_e4m3)

    # w-folded labels: label if label2==1 else C (one-hot row all-zero)
    wlab = np.full(bpad, float(C), dtype=np.float32)
    wlab[:b] = np.where(label2 == 1, labels, C).astype(np.float32)

    ctilT = (centers.T * (-2.0 / (XSCALE * XSCALE)) * XSCALE).astype(
        ml_dtypes.float8_e4m3
    )  # [256, 5] = -2*C^T / XSCALE (pairs with x*XSCALE)
    cf8 = np.ascontiguousarray(
        np.concatenate([ctilT[0:128], ctilT[128:256]], axis=1)
    )  # [128, 10]
    cnorm = (centers * centers).sum(axis=1).astype(np.float32)  # [5]
    iota = np.tile(
        np.arange(C, dtype=np.float32)[None, :], (128, 1)
    ).astype(ml_dtypes.bfloat16)

    in_maps = []
    for i in range(n_cores):
        sl = slice(i * rpc, (i + 1) * rpc)
        # XT layout [dpart, dchunk, tile, row]
        xi = np.ascontiguousarray(
            x_f8[sl].reshape(tiles, 128, 2, 128).transpose(3, 2, 0, 1)
        )
        # d2add[r, t*K+k] = norm2[row] + cnorm[k]
        n2 = norm2[sl].reshape(tiles, 128).T  # [128, tiles]
        d2add = np.ascontiguousarray(
            (n2[:, :, None] + cnorm[None, None, :]).reshape(128, tiles * K)
        )
        li = wlab[sl].reshape(tiles, 128).T.astype(ml_dtypes.bfloat16)
        cbf = np.ascontiguousarray(np.concatenate([iota, li], axis=1))
        in_maps.append({"x": xi, "const": d2add, "cbf": cbf, "cf8": cf8})
    return in_maps


def _host_final(parts, labels, label2, num_classes):
    """Sum per-core [77,112] partials, extract diagonal [11,16] blocks, and
    do the per-class argmin + final reduction (mirrors the reference)."""
    S = np.zeros((SEGP * NM, SEGP * C), dtype=np.float64)
    for p in parts:
        S += np.asarray(p, dtype=np.float64)
    seg = np.zeros((NM, C), dtype=np.float64)
    for p in range(SEGP):
        seg += S[p * NM:(p + 1) * NM, p * C:(p + 1) * C]

    labels = np.asarray(labels).astype(np.int64)
    label2 = np.asarray(label2)
    Ci = int(num_classes)
    w = (label2 == 1)
    cnt = np.bincount(labels[w], minlength=Ci).astype(np.float64)[:C]
    present = np.bincount(labels, minlength=Ci)[:C] > 0

    safe = np.maximum(cnt, 1.0)
    meand = seg[0:K] / safe[None, :]          # [K, C]
    closest = np.argmin(meand, axis=0)        # [C]
    sv = seg[K + closest, np.arange(C)]       # selected viol sums
    has = (cnt > 0).astype(np.float64)
    per_class = (seg[10] + sv) / safe * has
    n_unique = max(float(present.sum()), 1.0)
    return np.float32(per_class.sum() / n_unique)


_NC_CACHE = {}


def kernel(feat_normed, labels, label2, num_classes, centers, _trace=False):
    if "nc" not in _NC_CACHE:
        _NC_CACHE["nc"] = build_nc()
    nc = _NC_CACHE["nc"]
    in_maps = _host_prep(feat_normed, labels, label2, centers)
    res = run_bass_kernel_spmd(
        nc, in_maps, core_ids=list(range(NCORES)), trace=_trace
    )
    parts = [r["out"] for r in res.results]
    out = _host_final(parts, labels, label2, num_classes)
    if _trace:
        kernel.last_result = res
    return np.asarray(out, dtype=np.float32)


# revision 4
# speedup vs baseline: 3.4210x; 1.0311x over previous
"""DynamicSoftKMeansLoss on 8 Trainium2 NeuronCores.

Strategy (data-parallel over B, hardcoded for B=200000, D=256, K=5, C=16):
  - Host pads B to 8*25088 rows, shards across 8 cores, pre-transposes each
    shard to partition-major XT [128 dpart, 2 dchunk, T tiles, 128 rows] and
    casts to fp8 e4m3 (x scaled by 16, centers by 1/16 so the PE product is
    unscaled). Host also precomputes per-row |x|^2 (exact, f32), the combined
    d2add[r,t,k] = |x_rt|^2 + |c_k|^2 table, w-folded labels
    wlab = label if label2==1 else C (so the one-hot is w-weighted for free),
    and the per-class counts/presence (needed only by the final scalar math).
  - Per G-batch on device (uneven batches 21/77/77/21 tiles to shrink
    pipeline fill+drain): 2 fp8 matmuls per tile -> -2 x.c in PSUM;
    dist = exp(0.5*ln(psum + d2add)) written straight into vals as bf16;
    softmax weighted dist wd; min/2nd-min over the 5 centers;
    viol_j = relu(wd + margin - min_{k!=j} d_k) built as v1 + mask*(v2-v1).
    Elementwise [*, K] ops run in bf16 for 2x DVE rate; per-row stats in f32.
  - All per-class reductions are packed accumulating matmuls:
    psum[77,112] += vals7^T @ oh7 where 7 tiles' [128,11] vals / [128,16]
    one-hots are fused into one PE op (11 metrics: dist(5) | viol(5) | wd^2,
    all w-weighted via the one-hot).
  - Each core DMAs its [77,112] partial to DRAM; host sums the 8 partials,
    extracts the 7 diagonal [11,16] blocks, and does the tiny per-class
    argmin + final reduction in numpy (replaces a ~55us on-device
    collective+final-stage tail).
"""

import sys

sys.path.insert(0, "/opt/trn_rl_repo")

import numpy as np

import concourse.bass as bass
import concourse.bacc as bacc
import concourse.tile as tile
from concourse import mybir
from concourse.bass_utils import run_bass_kernel_spmd

F32 = mybir.dt.float32
BF16 = mybir.dt.bfloat16
F8 = mybir.dt.float8e4
ALU = mybir.AluOpType
ACTF = mybir.ActivationFunctionType
AX = mybir.AxisListType

B, D, K, C = 200000, 256, 5, 16
NCORES = 8
MARGIN = 0.5
BIG = float(2.0**40)

TILES = 196          # 196*128 = 25088 rows/core; 8*25088 = 200704 >= 200000
RPC = TILES * 128
GBS = (21, 77, 77, 21)   # per-G-batch tile counts (sum 196, all % 7 == 0)
NM = 11              # vals metrics: w*dist(5) | w*viol(5) | w*wd2
SEGP = 7             # tiles packed per segment matmul
XSCALE = 16.0        # fp8 range centering: x*16 on host, centers/16


def _b0(ap, n, axis="inner"):
    """Stride-0 broadcast of a 2D [128, G] (or [128, C]) AP to 3D."""
    pairs = [list(p) for p in ap.ap]
    if axis == "inner":
        newap = pairs + [[0, n]]
    else:  # outer: [128, C] -> [128, n, C]
        newap = [pairs[0], [0, n], pairs[1]]
    return bass.AP(tensor=ap.tensor, offset=ap.offset, ap=newap)


def _patch_act_tables():
    """Placement-only hint: hide Ln/Exp from every table except the combined
    natural_log_exp_and_others so Bacc's greedy table-load placement picks the
    one table that serves Ln, Exp and Relu together (ids stay valid)."""
    import concourse.bacc as _bacc
    from concourse.hw_specs import get_activation_tables as _orig

    def patched(arch):
        tabs = _orig(arch)
        keep = "natural_log_exp_and_others"
        if keep in tabs:
            for name, funcs in tabs.items():
                if name != keep:
                    funcs.discard(ACTF.Ln)
                    funcs.discard(ACTF.Exp)
        return tabs

    _bacc.get_activation_tables = patched


def build_nc(tiles=TILES, n_cores=NCORES):
    _patch_act_tables()
    nc = bacc.Bacc(None, num_devices=n_cores)
    assert sum(GBS) == tiles and all(g % SEGP == 0 for g in GBS)

    # host-pretransposed, fp8-cast XT layout: [dpart, dchunk, tile, row]
    x_dram = nc.declare_dram_parameter("x", [128, 2, tiles, 128], F8, isOutput=False)
    # f32 consts: d2add [128, tiles*K]
    const_dram = nc.declare_dram_parameter(
        "const", [128, tiles * K], F32, isOutput=False
    )
    # bf16 consts: iota [128, C] | wlab [128, tiles]
    cbf_dram = nc.declare_dram_parameter("cbf", [128, C + tiles], BF16, isOutput=False)
    # fp8 consts: ctil (two 128-chunks of -2*C^T/XSCALE)
    cf8_dram = nc.declare_dram_parameter("cf8", [128, 2 * K], F8, isOutput=False)
    out_dram = nc.declare_dram_parameter(
        "out", [SEGP * NM, SEGP * C], F32, isOutput=True
    )

    with tile.TileContext(nc) as tc:
        with (
            tc.tile_pool(name="consts", bufs=1) as consts,
            tc.tile_pool(name="xin", bufs=3) as xin,
            tc.tile_pool(name="big", bufs=2) as big,
            tc.tile_pool(name="stat", bufs=2) as stat,
            tc.tile_pool(name="ps_d", bufs=2, space="PSUM") as psd_pool,
            tc.tile_pool(name="ps_seg", bufs=1, space="PSUM") as psseg,
        ):
            # tiny matmul const first so the first PE op is never DMA-gated
            cf8_sb = consts.tile([128, 2 * K], F8)
            nc.sync.dma_start(cf8_sb[:], cf8_dram[:])
            const_sb = consts.tile([128, tiles * K], F32)
            nc.scalar.dma_start(const_sb[:], const_dram[:])
            cbf_sb = consts.tile([128, C + tiles], BF16)
            nc.scalar.dma_start(cbf_sb[:], cbf_dram[:])

            iota_sb = cbf_sb[:, 0:C]
            wlab_sb = cbf_sb[:, C:C + tiles]

            psum_seg = psseg.tile([SEGP * NM, SEGP * C], F32)

            t_off = 0
            p_off = 0
            for b, gb in enumerate(GBS):
                t0, t1 = t_off, t_off + gb
                t_off = t1
                xb = xin.tile([128, 2, gb, 128], F8)
                # per-chunk DMAs: chunk-0 matmuls start after half the load
                nc.sync.dma_start(xb[:, 0], x_dram[:, 0, t0:t1, :])
                nc.sync.dma_start(xb[:, 1], x_dram[:, 1, t0:t1, :])

                psd = psd_pool.tile([128, gb, K], F32)
                for c in range(2):
                    for g in range(gb):
                        nc.tensor.matmul(
                            psd[:, g, :], xb[:, c, g, :],
                            cf8_sb[:, c * K:(c + 1) * K],
                            start=(c == 0), stop=(c == 1),
                        )

                # w-weighted one-hot via stride-0 broadcasts
                oh = big.tile([128, gb, C], BF16, tag="oh")
                nc.vector.tensor_tensor(
                    oh[:], _b0(iota_sb, gb, "outer"),
                    _b0(wlab_sb[:, t0:t1], C, "inner"),
                    ALU.is_equal,
                )

                # d2 = psum + (|x|^2 + |c|^2); dist = exp(0.5*ln(d2)) lands
                # in vals as bf16 (Ln, Exp, Relu share one activation table)
                t_d2 = big.tile([128, gb, K], F32, tag="t_d2")
                nc.vector.tensor_tensor(
                    t_d2[:], psd[:],
                    const_sb[:, t0 * K:t1 * K].rearrange("p (g k) -> p g k", k=K),
                    ALU.add,
                )
                lnt = big.tile([128, gb, K], F32, tag="lnt")
                nc.scalar.activation(lnt[:], t_d2[:], ACTF.Ln)
                vals = big.tile([128, gb, NM], BF16, tag="vals")
                nc.scalar.activation(vals[:, :, 0:K], lnt[:], ACTF.Exp, scale=0.5)
                dist = vals[:, :, 0:K]

                # min and masked 2nd-min over the 5 centers
                m1 = stat.tile([128, gb], F32, tag="m1")
                nc.vector.tensor_reduce(m1[:], dist, axis=AX.X, op=ALU.min)
                mask = big.tile([128, gb, K], BF16, tag="mask")
                nc.vector.tensor_tensor(
                    mask[:], dist, _b0(m1[:], K), ALU.is_equal
                )
                dmask = big.tile([128, gb, K], BF16, tag="dmask")
                nc.vector.scalar_tensor_tensor(
                    dmask[:], mask[:], BIG, dist, ALU.mult, ALU.add
                )
                m2 = stat.tile([128, gb], F32, tag="m2")
                nc.vector.tensor_reduce(m2[:], dmask[:], axis=AX.X, op=ALU.min)

                # softmax-weighted distance wd (unnormalized exp is fine;
                # the max-subtraction cancels in the ratio)
                eu = big.tile([128, gb, K], BF16, tag="eu")
                nc.scalar.activation(eu[:], dist, ACTF.Exp, scale=-1.0)
                s = stat.tile([128, gb], F32, tag="s")
                nc.vector.tensor_reduce(s[:], eu[:], axis=AX.X, op=ALU.add)
                prod = big.tile([128, gb, K], BF16, tag="prod")
                nc.vector.tensor_tensor(prod[:], eu[:], dist, ALU.mult)
                spd = stat.tile([128, gb], F32, tag="spd")
                nc.vector.tensor_reduce(spd[:], prod[:], axis=AX.X, op=ALU.add)
                rs = stat.tile([128, gb], F32, tag="rs")
                nc.vector.reciprocal(rs[:], s[:])
                wd = stat.tile([128, gb], F32, tag="wd")
                nc.vector.tensor_tensor(wd[:], spd[:], rs[:], ALU.mult)

                wd3 = wd[:].rearrange("p (g o) -> p g o", o=1)
                nc.vector.tensor_tensor(vals[:, :, 10:11], wd3, wd3, ALU.mult)

                # viol_j = relu(wd+m - min_{k!=j} d_k) = v1 + mask*(v2-v1)
                t1s = stat.tile([128, gb], F32, tag="t1")
                nc.vector.scalar_tensor_tensor(
                    t1s[:], wd[:], MARGIN, m1[:], ALU.add, ALU.subtract
                )
                t2s = stat.tile([128, gb], F32, tag="t2")
                nc.vector.scalar_tensor_tensor(
                    t2s[:], wd[:], MARGIN, m2[:], ALU.add, ALU.subtract
                )
                v1 = stat.tile([128, gb], F32, tag="v1")
                nc.scalar.activation(v1[:], t1s[:], ACTF.Relu)
                v2 = stat.tile([128, gb], F32, tag="v2")
                nc.scalar.activation(v2[:], t2s[:], ACTF.Relu)
                dv = stat.tile([128, gb], F32, tag="dv")
                nc.vector.tensor_tensor(dv[:], v2[:], v1[:], ALU.subtract)
                vtmp = big.tile([128, gb, K], F32, tag="vtmp")
                nc.vector.tensor_tensor(vtmp[:], mask[:], _b0(dv[:], K), ALU.mult)
                nc.vector.tensor_tensor(
                    vals[:, :, K:2 * K], vtmp[:], _b0(v1[:], K), ALU.add
                )

                # segment accumulate, 7 tiles per PE op:
                # psum[77, 112] += vals[:, 7p:7p+7, :]^T @ oh[:, 7p:7p+7, :]
                npk = gb // SEGP
                for p in range(npk):
                    nc.tensor.matmul(
                        psum_seg[:],
                        vals[:, p * SEGP:(p + 1) * SEGP, :].rearrange(
                            "p g m -> p (g m)"
                        ),
                        oh[:, p * SEGP:(p + 1) * SEGP, :].rearrange(
                            "p g c -> p (g c)"
                        ),
                        start=(p_off + p == 0),
                        stop=(p_off + p == tiles // SEGP - 1),
                    )
                p_off += npk

            seg_sb = consts.tile([SEGP * NM, SEGP * C], F32, tag="seg_sb")
            nc.vector.tensor_copy(seg_sb[:], psum_seg[:])
            nc.sync.dma_start(out_dram[:], seg_sb[:])

    nc.compile()
    return nc


def _host_prep(feat, labels, label2, centers, tiles=TILES, n_cores=NCORES):
    """Pad + shard + pre-transpose + fp8-cast to per-core input maps."""
    import ml_dtypes

    rpc = tiles * 128
    bpad = rpc * n_cores
    b = feat.shape[0]

    feat = np.asarray(feat, dtype=np.float32)
    labels = np.asarray(labels)
    label2 = np.asarray(label2)
    centers = np.asarray(centers, dtype=np.float32)

    xpad = np.zeros((bpad, D), dtype=np.float32)
    xpad[:b] = feat
    norm2 = np.einsum("ij,ij->i", xpad, xpad, dtype=np.float32)
    x_f8 = (xpad * XSCALE).astype(ml_dtypes.float8_e4m3)

    # w-folded labels: label if label2==1 else C (one-hot row all-zero)
    wlab = np.full(bpad, float(C), dtype=np.float32)
    wlab[:b] = np.where(label2 == 1, labels, C).astype(np.float32)

    ctilT = (centers.T * (-2.0 / XSCALE)).astype(
        ml_dtypes.float8_e4m3
    )  # [256, 5]; pairs with x*XSCALE so the PE product is -2*x.c
    cf8 = np.ascontiguousarray(
        np.concatenate([ctilT[0:128], ctilT[128:256]], axis=1)
    )  # [128, 10]
    cnorm = (centers * centers).sum(axis=1).astype(np.float32)  # [5]
    iota = np.tile(
        np.arange(C, dtype=np.float32)[None, :], (128, 1)
    ).astype(ml_dtypes.bfloat16)

    in_maps = []
    for i in range(n_cores):
        sl = slice(i * rpc, (i + 1) * rpc)
        # XT layout [dpart, dchunk, tile, row]
        xi = np.ascontiguousarray(
            x_f8[sl].reshape(tiles, 128, 2, 128).transpose(3, 2, 0, 1)
        )
        # d2add[r, t*K+k] = norm2[row] + cnorm[k]
        n2 = norm2[sl].reshape(tiles, 128).T  # [128, tiles]
        d2add = np.ascontiguousarray(
            (n2[:, :, None] + cnorm[None, None, :]).reshape(128, tiles * K)
        )
        li = wlab[sl].reshape(tiles, 128).T.astype(ml_dtypes.bfloat16)
        cbf = np.ascontiguousarray(np.concatenate([iota, li], axis=1))
        in_maps.append({"x": xi, "const": d2add, "cbf": cbf, "cf8": cf8})
    return in_maps


def _host_final(parts, labels, label2, num_classes):
    """Sum per-core [77,112] partials, extract diagonal [11,16] blocks, and
    do the per-class argmin + final reduction (mirrors the reference)."""
    S = np.zeros((SEGP * NM, SEGP * C), dtype=np.float64)
    for p in parts:
        S += np.asarray(p, dtype=np.float64)
    seg = np.zeros((NM, C), dtype=np.float64)
    for p in range(SEGP):
        seg += S[p * NM:(p + 1) * NM, p * C:(p + 1) * C]

    labels = np.asarray(labels).astype(np.int64)
    label2 = np.asarray(label2)
    Ci = int(num_classes)
    w = (label2 == 1)
    cnt = np.bincount(labels[w], minlength=Ci).astype(np.float64)[:C]
    present = np.bincount(labels, minlength=Ci)[:C] > 0

    safe = np.maximum(cnt, 1.0)
    meand = seg[0:K] / safe[None, :]          # [K, C]
    closest = np.argmin(meand, axis=0)        # [C]
    sv = seg[K + closest, np.arange(C)]       # selected viol sums
    has = (cnt > 0).astype(np.float64)
    per_class = (seg[10] + sv) / safe * has
    n_unique = max(float(present.sum()), 1.0)
    return np.float32(per_class.sum() / n_unique)


_NC_CACHE = {}


def kernel(feat_normed, labels, label2, num_classes, centers, _trace=False):
    if "nc" not in _NC_CACHE:
        _NC_CACHE["nc"] = build_nc()
    nc = _NC_CACHE["nc"]
    in_maps = _host_prep(feat_normed, labels, label2, centers)
    res = run_bass_kernel_spmd(
        nc, in_maps, core_ids=list(range(NCORES)), trace=_trace
    )
    parts = [r["out"] for r in res.results]
    out = _host_final(parts, labels, label2, num_classes)
    if _trace:
        kernel.last_result = res
    return np.asarray(out, dtype=np.float32)


# revision 5
# speedup vs baseline: 4.3419x; 1.2692x over previous
"""DynamicSoftKMeansLoss on 8 Trainium2 NeuronCores.

Strategy (data-parallel over B, hardcoded for B=200000, D=256, K=5, C=16):
  - The loss depends on feat rows ONLY where label2==1 (every segment sum is
    w-weighted; the presence/count terms are host-side bincounts), so the
    host filters to those ~B/2 rows first — halving all device work.
  - Host pads the filtered rows to 8*T*128, shards across 8 cores,
    pre-transposes each shard to partition-major XT [128 dpart, 2 dchunk,
    T tiles, 128 rows] and casts to fp8 e4m3 (x scaled by 16, centers by
    1/16 so the PE product is unscaled). Host also precomputes per-row
    |x|^2 (exact, f32), the combined d2add[r,t,k] = |x|^2 + |c_k|^2 table,
    and labels padded with C (so the padded one-hot rows are all-zero).
  - Per G-batch on device: 2 fp8 matmuls per tile -> -2 x.c in PSUM;
    dist = exp(0.5*ln(psum + d2add)) written straight into vals as bf16;
    softmax weighted dist wd; min/2nd-min over the 5 centers;
    viol_j = relu(wd + margin - min_{k!=j} d_k) = relu(t1 - mask_j*(m2-m1)).
    The loop is software-pipelined: batch b+1's PSUM add + Ln/Exp run ahead
    of batch b's long DVE chain so Vector/Scalar/PE overlap across batches.
  - All per-class reductions are packed accumulating matmuls:
    psum[77,112] += vals7^T @ oh7 where 7 tiles' [128,11] vals / [128,16]
    one-hots are fused into one PE op (11 metrics: dist(5) | viol(5) | wd^2).
  - Each core DMAs its [77,112] partial to DRAM; host sums the 8 partials,
    extracts the 7 diagonal [11,16] blocks, and does the tiny per-class
    argmin + final reduction in numpy (replaces a ~55us on-device
    collective+final-stage tail).
"""

import sys

sys.path.insert(0, "/opt/trn_rl_repo")

import numpy as np

import concourse.bass as bass
import concourse.bacc as bacc
import concourse.tile as tile
from concourse import mybir
from concourse.bass_utils import run_bass_kernel_spmd

F32 = mybir.dt.float32
BF16 = mybir.dt.bfloat16
F8 = mybir.dt.float8e4
ALU = mybir.AluOpType
ACTF = mybir.ActivationFunctionType
AX = mybir.AxisListType

B, D, K, C = 200000, 256, 5, 16
NCORES = 8
MARGIN = 0.5
BIG = float(2.0**40)

NM = 11              # vals metrics: w*dist(5) | w*viol(5) | w*wd2
SEGP = 7             # tiles packed per segment matmul
XSCALE = 16.0        # fp8 range centering: x*16 on host, centers/16


def _batches(tiles):
    """Split tiles into G-batches (multiples of SEGP, small first batch)."""
    assert tiles % SEGP == 0
    if tiles <= 14:
        return [tiles]
    bs = [14]
    rem = tiles - 14
    while rem:
        c = min(28, rem)
        bs.append(c)
        rem -= c
    return bs


def _b0(ap, n, axis="inner"):
    """Stride-0 broadcast of a 2D [128, G] (or [128, C]) AP to 3D."""
    pairs = [list(p) for p in ap.ap]
    if axis == "inner":
        newap = pairs + [[0, n]]
    else:  # outer: [128, C] -> [128, n, C]
        newap = [pairs[0], [0, n], pairs[1]]
    return bass.AP(tensor=ap.tensor, offset=ap.offset, ap=newap)


def _patch_act_tables():
    """Placement-only hint: hide Ln/Exp from every table except the combined
    natural_log_exp_and_others so Bacc's greedy table-load placement picks the
    one table that serves Ln and Exp together (ids stay valid)."""
    import concourse.bacc as _bacc
    from concourse.hw_specs import get_activation_tables as _orig

    def patched(arch):
        tabs = _orig(arch)
        keep = "natural_log_exp_and_others"
        if keep in tabs:
            for name, funcs in tabs.items():
                if name != keep:
                    funcs.discard(ACTF.Ln)
                    funcs.discard(ACTF.Exp)
        return tabs

    _bacc.get_activation_tables = patched


def build_nc(tiles, n_cores=NCORES):
    _patch_act_tables()
    nc = bacc.Bacc(None, num_devices=n_cores)
    gbs = _batches(tiles)
    nb = len(gbs)

    x_dram = nc.declare_dram_parameter("x", [128, 2, tiles, 128], F8, isOutput=False)
    const_dram = nc.declare_dram_parameter(
        "const", [128, tiles * K], F32, isOutput=False
    )
    cbf_dram = nc.declare_dram_parameter("cbf", [128, C + tiles], BF16, isOutput=False)
    cf8_dram = nc.declare_dram_parameter("cf8", [128, 2 * K], F8, isOutput=False)
    out_dram = nc.declare_dram_parameter(
        "out", [SEGP * NM, SEGP * C], F32, isOutput=True
    )

    with tile.TileContext(nc) as tc:
        with (
            tc.tile_pool(name="consts", bufs=1) as consts,
            tc.tile_pool(name="xin", bufs=3) as xin,
            tc.tile_pool(name="big", bufs=2) as big,
            tc.tile_pool(name="stat", bufs=2) as stat,
            tc.tile_pool(name="ps_d", bufs=2, space="PSUM") as psd_pool,
            tc.tile_pool(name="ps_seg", bufs=1, space="PSUM") as psseg,
        ):
            cf8_sb = consts.tile([128, 2 * K], F8)
            nc.sync.dma_start(cf8_sb[:], cf8_dram[:])
            const_sb = consts.tile([128, tiles * K], F32)
            nc.scalar.dma_start(const_sb[:], const_dram[:])
            cbf_sb = consts.tile([128, C + tiles], BF16)
            nc.scalar.dma_start(cbf_sb[:], cbf_dram[:])

            iota_sb = cbf_sb[:, 0:C]
            wlab_sb = cbf_sb[:, C:C + tiles]

            psum_seg = psseg.tile([SEGP * NM, SEGP * C], F32)

            offs = np.cumsum([0] + gbs)
            st = [dict() for _ in range(nb)]

            def emit_load_mm(b):
                gb, t0, t1 = gbs[b], offs[b], offs[b + 1]
                xb = xin.tile([128, 2, gb, 128], F8, tag="xb")
                nc.sync.dma_start(xb[:, 0], x_dram[:, 0, t0:t1, :])
                nc.sync.dma_start(xb[:, 1], x_dram[:, 1, t0:t1, :])
                psd = psd_pool.tile([128, gb, K], F32, tag="psd")
                for c in range(2):
                    for g in range(gb):
                        nc.tensor.matmul(
                            psd[:, g, :], xb[:, c, g, :],
                            cf8_sb[:, c * K:(c + 1) * K],
                            start=(c == 0), stop=(c == 1),
                        )
                st[b]["psd"] = psd

            def emit_a(b):
                """PSUM readout + ACT chain + one-hot for batch b (runs ahead
                of batch b-1's DVE chain)."""
                gb, t0, t1 = gbs[b], offs[b], offs[b + 1]
                t_d2 = big.tile([128, gb, K], F32, tag="t_d2")
                nc.vector.tensor_tensor(
                    t_d2[:], st[b]["psd"][:],
                    const_sb[:, t0 * K:t1 * K].rearrange("p (g k) -> p g k", k=K),
                    ALU.add,
                )
                lnt = big.tile([128, gb, K], F32, tag="lnt")
                nc.scalar.activation(lnt[:], t_d2[:], ACTF.Ln)
                vals = big.tile([128, gb, NM], BF16, tag="vals")
                nc.scalar.activation(vals[:, :, 0:K], lnt[:], ACTF.Exp, scale=0.5)
                eu = big.tile([128, gb, K], BF16, tag="eu")
                nc.scalar.activation(eu[:], vals[:, :, 0:K], ACTF.Exp, scale=-1.0)
                oh = big.tile([128, gb, C], BF16, tag="oh")
                nc.vector.tensor_tensor(
                    oh[:], _b0(iota_sb, gb, "outer"),
                    _b0(wlab_sb[:, t0:t1], C, "inner"),
                    ALU.is_equal,
                )
                st[b]["vals"], st[b]["eu"], st[b]["oh"] = vals, eu, oh

            def emit_b(b):
                """Main DVE chain + packed segment matmuls for batch b."""
                gb = gbs[b]
                vals, eu, oh = st[b]["vals"], st[b]["eu"], st[b]["oh"]
                dist = vals[:, :, 0:K]

                m1 = stat.tile([128, gb], F32, tag="m1")
                nc.vector.tensor_reduce(m1[:], dist, axis=AX.X, op=ALU.min)
                mask = big.tile([128, gb, K], BF16, tag="mask")
                nc.vector.tensor_tensor(
                    mask[:], dist, _b0(m1[:], K), ALU.is_equal
                )
                dmask = big.tile([128, gb, K], BF16, tag="dmask")
                nc.vector.scalar_tensor_tensor(
                    dmask[:], mask[:], BIG, dist, ALU.mult, ALU.add
                )
                m2 = stat.tile([128, gb], F32, tag="m2")
                nc.vector.tensor_reduce(m2[:], dmask[:], axis=AX.X, op=ALU.min)

                s = stat.tile([128, gb], F32, tag="s")
                nc.vector.tensor_reduce(s[:], eu[:], axis=AX.X, op=ALU.add)
                prod = big.tile([128, gb, K], BF16, tag="prod")
                nc.vector.tensor_tensor(prod[:], eu[:], dist, ALU.mult)
                spd = stat.tile([128, gb], F32, tag="spd")
                nc.vector.tensor_reduce(spd[:], prod[:], axis=AX.X, op=ALU.add)
                rs = stat.tile([128, gb], F32, tag="rs")
                nc.vector.reciprocal(rs[:], s[:])
                wd = stat.tile([128, gb], F32, tag="wd")
                nc.vector.tensor_tensor(wd[:], spd[:], rs[:], ALU.mult)

                wd3 = wd[:].rearrange("p (g o) -> p g o", o=1)
                nc.vector.tensor_tensor(vals[:, :, 10:11], wd3, wd3, ALU.mult)

                # viol_j = relu(t1 - mask_j*(m2-m1)), t1 = wd + margin - m1
                dl = stat.tile([128, gb], F32, tag="dl")
                nc.vector.tensor_tensor(dl[:], m2[:], m1[:], ALU.subtract)
                t1s = stat.tile([128, gb], F32, tag="t1")
                nc.vector.scalar_tensor_tensor(
                    t1s[:], wd[:], MARGIN, m1[:], ALU.add, ALU.subtract
                )
                mdl = big.tile([128, gb, K], F32, tag="mdl")
                nc.vector.tensor_tensor(mdl[:], mask[:], _b0(dl[:], K), ALU.mult)
                harg = big.tile([128, gb, K], F32, tag="harg")
                nc.vector.scalar_tensor_tensor(
                    harg[:], mdl[:], -1.0, _b0(t1s[:], K), ALU.mult, ALU.add
                )
                nc.vector.tensor_scalar(
                    vals[:, :, K:2 * K], harg[:], 0.0, None, ALU.max
                )

                npk = gb // SEGP
                p_base = offs[b] // SEGP
                for p in range(npk):
                    nc.tensor.matmul(
                        psum_seg[:],
                        vals[:, p * SEGP:(p + 1) * SEGP, :].rearrange(
                            "p g m -> p (g m)"
                        ),
                        oh[:, p * SEGP:(p + 1) * SEGP, :].rearrange(
                            "p g c -> p (g c)"
                        ),
                        start=(p_base + p == 0),
                        stop=(p_base + p == tiles // SEGP - 1),
                    )

            # software pipeline: A(b+1) is emitted before B(b) so the PSUM
            # readout + Ln/Exp of the next batch overlap the current DVE chain
            emit_load_mm(0)
            emit_a(0)
            for b in range(nb):
                if b + 1 < nb:
                    emit_load_mm(b + 1)
                    emit_a(b + 1)
                emit_b(b)

            seg_sb = consts.tile([SEGP * NM, SEGP * C], F32, tag="seg_sb")
            nc.vector.tensor_copy(seg_sb[:], psum_seg[:])
            nc.sync.dma_start(out_dram[:], seg_sb[:])

    nc.compile()
    return nc


def _host_prep(feat, labels, label2, centers, tiles, n_cores=NCORES):
    """Filter w==1 rows, pad + shard + pre-transpose + fp8-cast."""
    import ml_dtypes

    rpc = tiles * 128
    bpad = rpc * n_cores

    feat = np.asarray(feat, dtype=np.float32)
    labels = np.asarray(labels)
    label2 = np.asarray(label2)
    centers = np.asarray(centers, dtype=np.float32)

    idx = np.flatnonzero(label2 == 1)
    nw = idx.size

    xpad = np.zeros((bpad, D), dtype=np.float32)
    xpad[:nw] = feat[idx]
    norm2 = np.einsum("ij,ij->i", xpad, xpad, dtype=np.float32)
    x_f8 = (xpad * XSCALE).astype(ml_dtypes.float8_e4m3)

    wlab = np.full(bpad, float(C), dtype=np.float32)
    wlab[:nw] = labels[idx].astype(np.float32)

    ctilT = (centers.T * (-2.0 / XSCALE)).astype(ml_dtypes.float8_e4m3)
    cf8 = np.ascontiguousarray(
        np.concatenate([ctilT[0:128], ctilT[128:256]], axis=1)
    )  # [128, 10]
    cnorm = (centers * centers).sum(axis=1).astype(np.float32)  # [5]
    iota = np.tile(
        np.arange(C, dtype=np.float32)[None, :], (128, 1)
    ).astype(ml_dtypes.bfloat16)

    in_maps = []
    for i in range(n_cores):
        sl = slice(i * rpc, (i + 1) * rpc)
        xi = np.ascontiguousarray(
            x_f8[sl].reshape(tiles, 128, 2, 128).transpose(3, 2, 0, 1)
        )
        n2 = norm2[sl].reshape(tiles, 128).T  # [128, tiles]
        d2add = np.ascontiguousarray(
            (n2[:, :, None] + cnorm[None, None, :]).reshape(128, tiles * K)
        )
        li = wlab[sl].reshape(tiles, 128).T.astype(ml_dtypes.bfloat16)
        cbf = np.ascontiguousarray(np.concatenate([iota, li], axis=1))
        in_maps.append({"x": xi, "const": d2add, "cbf": cbf, "cf8": cf8})
    return in_maps


def _host_final(parts, labels, label2, num_classes):
    """Sum per-core [77,112] partials, extract diagonal [11,16] blocks, and
    do the per-class argmin + final reduction (mirrors the reference)."""
    seg = np.zeros((NM, C), dtype=np.float64)
    if parts:
        S = np.zeros((SEGP * NM, SEGP * C), dtype=np.float64)
        for p in parts:
            S += np.asarray(p, dtype=np.float64)
        for p in range(SEGP):
            seg += S[p * NM:(p + 1) * NM, p * C:(p + 1) * C]

    labels = np.asarray(labels).astype(np.int64)
    label2 = np.asarray(label2)
    Ci = int(num_classes)
    w = (label2 == 1)
    cnt = np.bincount(labels[w], minlength=Ci).astype(np.float64)[:C]
    present = np.bincount(labels, minlength=Ci)[:C] > 0

    safe = np.maximum(cnt, 1.0)
    meand = seg[0:K] / safe[None, :]          # [K, C]
    closest = np.argmin(meand, axis=0)        # [C]
    sv = seg[K + closest, np.arange(C)]       # selected viol sums
    has = (cnt > 0).astype(np.float64)
    per_class = (seg[10] + sv) / safe * has
    n_unique = max(float(present.sum()), 1.0)
    return np.float32(per_class.sum() / n_unique)


_NC_CACHE = {}


def kernel(feat_normed, labels, label2, num_classes, centers, _trace=False):
    label2 = np.asarray(label2)
    nw = int((label2 == 1).sum())
    if nw == 0:
        return np.asarray(
            _host_final([], labels, label2, num_classes), dtype=np.float32
        )
    tiles = -(-nw // (128 * NCORES))          # ceil rows / (128*cores)
    tiles = SEGP * (-(-tiles // SEGP))        # round up to multiple of SEGP
    if tiles not in _NC_CACHE:
        _NC_CACHE[tiles] = build_nc(tiles)
    nc = _NC_CACHE[tiles]
    in_maps = _host_prep(feat_normed, labels, label2, centers, tiles)
    res = run_bass_kernel_spmd(
        nc, in_maps, core_ids=list(range(NCORES)), trace=_trace
    )
    parts = [r["out"] for r in res.results]
    out = _host_final(parts, labels, label2, num_classes)
    if _trace:
        kernel.last_result = res
    return np.asarray(out, dtype=np.float32)


# revision 8
# speedup vs baseline: 4.8831x; 1.1246x over previous
"""DynamicSoftKMeansLoss on 8 Trainium2 NeuronCores.

Strategy (data-parallel over B, hardcoded for B=200000, D=256, K=5, C=16):
  - The loss depends on feat rows ONLY where label2==1 (every segment sum is
    w-weighted; the presence/count terms are host-side bincounts), so the
    host filters to those ~B/2 rows first — halving all device work.
  - Host pads the filtered rows to 8*T*128, shards across 8 cores,
    pre-transposes each shard to partition-major XT [128 dpart, 2 dchunk,
    T tiles, 128 rows] and casts to fp8 e4m3 (x scaled by 16, centers by
    1/16 so the PE product is unscaled). Host also precomputes per-row
    |x|^2 (exact, f32), the combined d2add[r,t,k] = |x|^2 + |c_k|^2 table,
    and the w-weighted one-hot labels (bf16, padded rows all-zero).
  - Per G-batch on device: 2 fp8 matmuls per tile -> -2 x.c in PSUM;
    dist = exp(0.5*ln(psum + d2add)) written straight into vals as bf16;
    softmax weighted dist wd; min/2nd-min over the 5 centers;
    viol_j = relu(wd + margin - min_{k!=j} d_k) = relu(t1 - mask_j*(m2-m1)).
    The loop is software-pipelined: batch b+1's PSUM add + Ln/Exp run ahead
    of batch b's long DVE chain so Vector/Scalar/PE overlap across batches.
    A block of dummy warmup matmuls at program start ramps the PE out of its
    low-frequency pstate before the first real tile arrives.
  - All per-class reductions are packed accumulating matmuls:
    psum[77,112] += vals7^T @ oh7 where 7 tiles' [128,11] vals / [128,16]
    one-hots are fused into one PE op (11 metrics: dist(5) | viol(5) | wd^2).
  - Each core DMAs its [77,112] partial to DRAM; host sums the 8 partials,
    extracts the 7 diagonal [11,16] blocks, and does the tiny per-class
    argmin + final reduction in numpy (replaces a ~55us on-device
    collective+final-stage tail).
"""

import sys

sys.path.insert(0, "/opt/trn_rl_repo")

import numpy as np

import concourse.bass as bass
import concourse.bacc as bacc
import concourse.tile as tile
from concourse import mybir
from concourse.bass_utils import run_bass_kernel_spmd

F32 = mybir.dt.float32
BF16 = mybir.dt.bfloat16
F8 = mybir.dt.float8e4
ALU = mybir.AluOpType
ACTF = mybir.ActivationFunctionType
AX = mybir.AxisListType

B, D, K, C = 200000, 256, 5, 16
NCORES = 8
MARGIN = 0.5
BIG = float(2.0**40)

NM = 11              # vals metrics: w*dist(5) | w*viol(5) | w*wd2
SEGP = 7             # tiles packed per segment matmul
XSCALE = 16.0        # fp8 range centering: x*16 on host, centers/16
NWARM = 40           # PE pstate warmup matmuls


def _batches(tiles):
    """Split tiles into G-batches (multiples of SEGP, small first batch)."""
    assert tiles % SEGP == 0
    if tiles <= 14:
        return [tiles]
    bs = [14]
    rem = tiles - 14
    while rem:
        c = min(42, rem)
        bs.append(c)
        rem -= c
    return bs


def _b0(ap, n, axis="inner"):
    """Stride-0 broadcast of a 2D [128, G] AP to 3D."""
    pairs = [list(p) for p in ap.ap]
    if axis == "inner":
        newap = pairs + [[0, n]]
    else:
        newap = [pairs[0], [0, n], pairs[1]]
    return bass.AP(tensor=ap.tensor, offset=ap.offset, ap=newap)


def _patch_act_tables():
    """Placement-only hint: hide Ln/Exp from every table except the combined
    natural_log_exp_and_others so Bacc's greedy table-load placement picks the
    one table that serves Ln and Exp together (ids stay valid)."""
    import concourse.bacc as _bacc
    from concourse.hw_specs import get_activation_tables as _orig

    def patched(arch):
        tabs = _orig(arch)
        keep = "natural_log_exp_and_others"
        if keep in tabs:
            for name, funcs in tabs.items():
                if name != keep:
                    funcs.discard(ACTF.Ln)
                    funcs.discard(ACTF.Exp)
        return tabs

    _bacc.get_activation_tables = patched


def build_nc(tiles, n_cores=NCORES):
    _patch_act_tables()
    nc = bacc.Bacc(None, num_devices=n_cores)
    gbs = _batches(tiles)
    nb = len(gbs)

    x_dram = nc.declare_dram_parameter("x", [128, 2, tiles, 128], F8, isOutput=False)
    const_dram = nc.declare_dram_parameter(
        "const", [128, tiles * K], F32, isOutput=False
    )
    oh_dram = nc.declare_dram_parameter("oh", [128, tiles, C], BF16, isOutput=False)
    cf8_dram = nc.declare_dram_parameter("cf8", [128, 2 * K], F8, isOutput=False)
    out_dram = nc.declare_dram_parameter(
        "out", [SEGP * NM, SEGP * C], F32, isOutput=True
    )

    with tile.TileContext(nc) as tc:
        with (
            tc.tile_pool(name="consts", bufs=1) as consts,
            tc.tile_pool(name="xin", bufs=3) as xin,
            tc.tile_pool(name="ohin", bufs=2) as ohin,
            tc.tile_pool(name="big", bufs=2) as big,
            tc.tile_pool(name="stat", bufs=2) as stat,
            tc.tile_pool(name="ps_d", bufs=2, space="PSUM") as psd_pool,
            tc.tile_pool(name="ps_seg", bufs=1, space="PSUM") as psseg,
            tc.tile_pool(name="ps_warm", bufs=1, space="PSUM") as pswarm,
        ):
            cf8_sb = consts.tile([128, 2 * K], F8)
            nc.sync.dma_start(cf8_sb[:], cf8_dram[:])
            const_sb = consts.tile([128, tiles * K], F32)
            nc.scalar.dma_start(const_sb[:], const_dram[:])

            # PE pstate warmup: keep the tensor engine continuously busy from
            # right after the tiny cf8 load until real tiles arrive
            warm_ps = pswarm.tile([2 * K, 2 * K], F32)
            for _ in range(NWARM):
                nc.tensor.matmul(
                    warm_ps[:], cf8_sb[:], cf8_sb[:], start=True, stop=True
                )

            psum_seg = psseg.tile([SEGP * NM, SEGP * C], F32)

            offs = np.cumsum([0] + gbs)
            st = [dict() for _ in range(nb)]

            def emit_load_mm(b):
                gb, t0, t1 = gbs[b], offs[b], offs[b + 1]
                xb = xin.tile([128, 2, gb, 128], F8, tag="xb")
                nc.sync.dma_start(xb[:, 0], x_dram[:, 0, t0:t1, :])
                nc.sync.dma_start(xb[:, 1], x_dram[:, 1, t0:t1, :])
                oh = ohin.tile([128, gb, C], BF16, tag="oh")
                nc.scalar.dma_start(oh[:], oh_dram[:, t0:t1, :])
                psd = psd_pool.tile([128, gb, K], F32, tag="psd")
                for c in range(2):
                    for g in range(gb):
                        nc.tensor.matmul(
                            psd[:, g, :], xb[:, c, g, :],
                            cf8_sb[:, c * K:(c + 1) * K],
                            start=(c == 0), stop=(c == 1),
                        )
                st[b]["psd"], st[b]["oh"] = psd, oh

            def emit_a(b):
                """PSUM readout + ACT chain for batch b (runs ahead of batch
                b-1's DVE chain)."""
                gb, t0, t1 = gbs[b], offs[b], offs[b + 1]
                t_d2 = big.tile([128, gb, K], F32, tag="t_d2")
                nc.vector.tensor_tensor(
                    t_d2[:], st[b]["psd"][:],
                    const_sb[:, t0 * K:t1 * K].rearrange("p (g k) -> p g k", k=K),
                    ALU.add,
                )
                lnt = big.tile([128, gb, K], F32, tag="lnt")
                nc.scalar.activation(lnt[:], t_d2[:], ACTF.Ln)
                vals = big.tile([128, gb, NM], BF16, tag="vals")
                nc.scalar.activation(vals[:, :, 0:K], lnt[:], ACTF.Exp, scale=0.5)
                eu = big.tile([128, gb, K], BF16, tag="eu")
                nc.scalar.activation(eu[:], vals[:, :, 0:K], ACTF.Exp, scale=-1.0)
                st[b]["vals"], st[b]["eu"] = vals, eu

            def emit_b(b):
                """Main DVE chain + packed segment matmuls for batch b."""
                gb = gbs[b]
                vals, eu, oh = st[b]["vals"], st[b]["eu"], st[b]["oh"]
                dist = vals[:, :, 0:K]

                m1 = stat.tile([128, gb], F32, tag="m1")
                nc.vector.tensor_reduce(m1[:], dist, axis=AX.X, op=ALU.min)
                mask = big.tile([128, gb, K], BF16, tag="mask")
                nc.vector.tensor_tensor(
                    mask[:], dist, _b0(m1[:], K), ALU.is_equal
                )
                dmask = big.tile([128, gb, K], BF16, tag="dmask")
                nc.vector.scalar_tensor_tensor(
                    dmask[:], mask[:], BIG, dist, ALU.mult, ALU.add
                )
                m2 = stat.tile([128, gb], F32, tag="m2")
                nc.vector.tensor_reduce(m2[:], dmask[:], axis=AX.X, op=ALU.min)

                s = stat.tile([128, gb], F32, tag="s")
                nc.vector.tensor_reduce(s[:], eu[:], axis=AX.X, op=ALU.add)
                prod = big.tile([128, gb, K], BF16, tag="prod")
                nc.vector.tensor_tensor(prod[:], eu[:], dist, ALU.mult)
                spd = stat.tile([128, gb], F32, tag="spd")
                nc.vector.tensor_reduce(spd[:], prod[:], axis=AX.X, op=ALU.add)
                rs = stat.tile([128, gb], F32, tag="rs")
                nc.vector.reciprocal(rs[:], s[:])
                wd = stat.tile([128, gb], F32, tag="wd")
                nc.vector.tensor_tensor(wd[:], spd[:], rs[:], ALU.mult)

                wd3 = wd[:].rearrange("p (g o) -> p g o", o=1)
                nc.vector.tensor_tensor(vals[:, :, 10:11], wd3, wd3, ALU.mult)

                # viol_j = relu(t1 - mask_j*(m2-m1)), t1 = wd + margin - m1
                dl = stat.tile([128, gb], F32, tag="dl")
                nc.vector.tensor_tensor(dl[:], m2[:], m1[:], ALU.subtract)
                t1s = stat.tile([128, gb], F32, tag="t1")
                nc.vector.scalar_tensor_tensor(
                    t1s[:], wd[:], MARGIN, m1[:], ALU.add, ALU.subtract
                )
                mdl = big.tile([128, gb, K], F32, tag="mdl")
                nc.vector.tensor_tensor(mdl[:], mask[:], _b0(dl[:], K), ALU.mult)
                harg = big.tile([128, gb, K], F32, tag="harg")
                nc.vector.scalar_tensor_tensor(
                    harg[:], mdl[:], -1.0, _b0(t1s[:], K), ALU.mult, ALU.add
                )
                nc.vector.tensor_scalar(
                    vals[:, :, K:2 * K], harg[:], 0.0, None, ALU.max
                )

                npk = gb // SEGP
                p_base = offs[b] // SEGP
                for p in range(npk):
                    nc.tensor.matmul(
                        psum_seg[:],
                        vals[:, p * SEGP:(p + 1) * SEGP, :].rearrange(
                            "p g m -> p (g m)"
                        ),
                        oh[:, p * SEGP:(p + 1) * SEGP, :].rearrange(
                            "p g c -> p (g c)"
                        ),
                        start=(p_base + p == 0),
                        stop=(p_base + p == tiles // SEGP - 1),
                    )

            # software pipeline: A(b+1) is emitted before B(b) so the PSUM
            # readout + Ln/Exp of the next batch overlap the current DVE chain
            emit_load_mm(0)
            emit_a(0)
            for b in range(nb):
                if b + 1 < nb:
                    emit_load_mm(b + 1)
                    emit_a(b + 1)
                emit_b(b)

            seg_sb = consts.tile([SEGP * NM, SEGP * C], F32, tag="seg_sb")
            nc.vector.tensor_copy(seg_sb[:], psum_seg[:])
            nc.sync.dma_start(out_dram[:], seg_sb[:])

    nc.compile()
    return nc


def _host_prep(feat, labels, label2, centers, tiles, n_cores=NCORES):
    """Filter w==1 rows, pad + shard + pre-transpose + fp8-cast."""
    import ml_dtypes

    rpc = tiles * 128
    bpad = rpc * n_cores

    feat = np.asarray(feat, dtype=np.float32)
    labels = np.asarray(labels)
    label2 = np.asarray(label2)
    centers = np.asarray(centers, dtype=np.float32)

    idx = np.flatnonzero(label2 == 1)
    nw = idx.size

    xpad = np.zeros((bpad, D), dtype=np.float32)
    xpad[:nw] = feat[idx]
    norm2 = np.einsum("ij,ij->i", xpad, xpad, dtype=np.float32)
    x_f8 = (xpad * XSCALE).astype(ml_dtypes.float8_e4m3)

    wlab = np.full(bpad, C, dtype=np.int64)
    wlab[:nw] = labels[idx]
    # one-hot [bpad, C] bf16; padded rows (class C) are all-zero
    oh_full = (wlab[:, None] == np.arange(C)[None, :]).astype(ml_dtypes.bfloat16)

    ctilT = (centers.T * (-2.0 / XSCALE)).astype(ml_dtypes.float8_e4m3)
    cf8 = np.ascontiguousarray(
        np.concatenate([ctilT[0:128], ctilT[128:256]], axis=1)
    )  # [128, 10]
    cnorm = (centers * centers).sum(axis=1).astype(np.float32)  # [5]

    in_maps = []
    for i in range(n_cores):
        sl = slice(i * rpc, (i + 1) * rpc)
        xi = np.ascontiguousarray(
            x_f8[sl].reshape(tiles, 128, 2, 128).transpose(3, 2, 0, 1)
        )
        n2 = norm2[sl].reshape(tiles, 128).T  # [128, tiles]
        d2add = np.ascontiguousarray(
            (n2[:, :, None] + cnorm[None, None, :]).reshape(128, tiles * K)
        )
        ohi = np.ascontiguousarray(
            oh_full[sl].reshape(tiles, 128, C).transpose(1, 0, 2)
        )
        in_maps.append({"x": xi, "const": d2add, "oh": ohi, "cf8": cf8})
    return in_maps


def _host_final(parts, labels, label2, num_classes):
    """Sum per-core [77,112] partials, extract diagonal [11,16] blocks, and
    do the per-class argmin + final reduction (mirrors the reference)."""
    seg = np.zeros((NM, C), dtype=np.float64)
    if parts:
        S = np.zeros((SEGP * NM, SEGP * C), dtype=np.float64)
        for p in parts:
            S += np.asarray(p, dtype=np.float64)
        for p in range(SEGP):
            seg += S[p * NM:(p + 1) * NM, p * C:(p + 1) * C]

    labels = np.asarray(labels).astype(np.int64)
    label2 = np.asarray(label2)
    Ci = int(num_classes)
    w = (label2 == 1)
    cnt = np.bincount(labels[w], minlength=Ci).astype(np.float64)[:C]
    present = np.bincount(labels, minlength=Ci)[:C] > 0

    safe = np.maximum(cnt, 1.0)
    meand = seg[0:K] / safe[None, :]          # [K, C]
    closest = np.argmin(meand, axis=0)        # [C]
    sv = seg[K + closest, np.arange(C)]       # selected viol sums
    has = (cnt > 0).astype(np.float64)
    per_class = (seg[10] + sv) / safe * has
    n_unique = max(float(present.sum()), 1.0)
    return np.float32(per_class.sum() / n_unique)


_NC_CACHE = {}


def kernel(feat_normed, labels, label2, num_classes, centers, _trace=False):
    label2 = np.asarray(label2)
    nw = int((label2 == 1).sum())
    if nw == 0:
        return np.asarray(
            _host_final([], labels, label2, num_classes), dtype=np.float32
        )
    tiles = -(-nw // (128 * NCORES))          # ceil rows / (128*cores)
    tiles = SEGP * (-(-tiles // SEGP))        # round up to multiple of SEGP
    if tiles not in _NC_CACHE:
        _NC_CACHE[tiles] = build_nc(tiles)
    nc = _NC_CACHE[tiles]
    in_maps = _host_prep(feat_normed, labels, label2, centers, tiles)
    res = run_bass_kernel_spmd(
        nc, in_maps, core_ids=list(range(NCORES)), trace=_trace
    )
    parts = [r["out"] for r in res.results]
    out = _host_final(parts, labels, label2, num_classes)
    if _trace:
        kernel.last_result = res
    return np.asarray(out, dtype=np.float32)


# revision 10
# speedup vs baseline: 4.9646x; 1.0167x over previous
"""DynamicSoftKMeansLoss on 8 Trainium2 NeuronCores.

Strategy (data-parallel over B, hardcoded for B=200000, D=256, K=5, C=16):
  - The loss depends on feat rows ONLY where label2==1 (every segment sum is
    w-weighted; the presence/count terms are host-side bincounts), so the
    host filters to those ~B/2 rows first — halving all device work.
  - Host pads the filtered rows to 8*T*128, shards across 8 cores,
    pre-transposes each shard to partition-major XT [128 dpart, 2 dchunk,
    T tiles, 128 rows] and casts to fp8 e4m3 (x scaled by 16, centers by
    1/16 so the PE product is unscaled). Host also precomputes the
    w-weighted one-hot labels (bf16, padded rows all-zero). feat_normed has
    unit rows by construction, so |x|^2 = 1 and d2 = 1 + |c_k|^2 - 2 x.c
    needs only a [K]-vector add (host-verified, with a per-row-table
    fallback build if rows are not unit-norm).
  - Per G-batch on device (sizes 7/14/21/28/28... so compute starts before
    the full x stream lands): 2 fp8 matmuls per tile -> -2 x.c in PSUM;
    dist = exp(0.5*ln(psum + d2add)) written straight into vals as bf16;
    softmax weighted dist wd; min/2nd-min over the 5 centers. Instead of
    materializing viol_j per row, vals carries v1 = relu(wd+m-min) and
    mask_j*(v2-v1) so the per-class argmin slot is resolved on the host:
    sum viol_{j*} = sum v1 + [sum mask_j*(v2-v1)]_{j*}.
    The loop is software-pipelined: batch b+1's PSUM add + Ln/Exp run ahead
    of batch b's DVE chain; dummy warmup matmuls ramp the PE pstate early.
  - All per-class reductions are packed accumulating matmuls:
    psum[84,112] += vals7^T @ oh7 where 7 tiles' [128,12] vals / [128,16]
    one-hots are fused into one PE op (12 metrics: dist(5) | v1 |
    mask*dv(5) | wd^2).
  - Each core DMAs its [84,112] partial to DRAM; host sums the 8 partials,
    extracts the 7 diagonal [12,16] blocks, and does the tiny per-class
    argmin + final reduction in numpy (replaces a ~55us on-device
    collective+final-stage tail).
"""

import sys

sys.path.insert(0, "/opt/trn_rl_repo")

import numpy as np

import concourse.bass as bass
import concourse.bacc as bacc
import concourse.tile as tile
from concourse import mybir
from concourse.bass_utils import run_bass_kernel_spmd

F32 = mybir.dt.float32
BF16 = mybir.dt.bfloat16
F8 = mybir.dt.float8e4
ALU = mybir.AluOpType
ACTF = mybir.ActivationFunctionType
AX = mybir.AxisListType

B, D, K, C = 200000, 256, 5, 16
NCORES = 8
MARGIN = 0.5
BIG = float(2.0**40)

NM = 12              # vals metrics: dist(5) | v1 | mask*dv(5) | wd^2
SEGP = 7             # tiles packed per segment matmul
XSCALE = 16.0        # fp8 range centering: x*16 on host, centers/16
NWARM = 64           # PE pstate warmup matmuls


def _batches(tiles):
    """Split tiles into G-batches (multiples of SEGP, ramped sizes so the
    first DVE chains start before the full x stream lands)."""
    assert tiles % SEGP == 0
    bs = []
    rem = tiles
    for want in (7, 14, 21):
        if rem <= 0:
            break
        c = min(want, rem)
        bs.append(c)
        rem -= c
    while rem:
        c = min(28, rem)
        bs.append(c)
        rem -= c
    return bs


def _b0(ap, n, axis="inner"):
    """Stride-0 broadcast of a 2D [128, G] AP to 3D."""
    pairs = [list(p) for p in ap.ap]
    if axis == "inner":
        newap = pairs + [[0, n]]
    else:
        newap = [pairs[0], [0, n], pairs[1]]
    return bass.AP(tensor=ap.tensor, offset=ap.offset, ap=newap)


def _patch_act_tables():
    """Placement-only hint: hide Ln/Exp from every table except the combined
    natural_log_exp_and_others so Bacc's greedy table-load placement picks the
    one table that serves Ln and Exp together (ids stay valid)."""
    import concourse.bacc as _bacc
    from concourse.hw_specs import get_activation_tables as _orig

    def patched(arch):
        tabs = _orig(arch)
        keep = "natural_log_exp_and_others"
        if keep in tabs:
            for name, funcs in tabs.items():
                if name != keep:
                    funcs.discard(ACTF.Ln)
                    funcs.discard(ACTF.Exp)
        return tabs

    _bacc.get_activation_tables = patched


def build_nc(tiles, unit_norm, n_cores=NCORES):
    _patch_act_tables()
    nc = bacc.Bacc(None, num_devices=n_cores)
    gbs = _batches(tiles)
    nb = len(gbs)

    x_dram = nc.declare_dram_parameter("x", [128, 2, tiles, 128], F8, isOutput=False)
    ncst = K if unit_norm else tiles * K
    const_dram = nc.declare_dram_parameter("const", [128, ncst], F32, isOutput=False)
    oh_dram = nc.declare_dram_parameter("oh", [128, tiles, C], BF16, isOutput=False)
    cf8_dram = nc.declare_dram_parameter("cf8", [128, 2 * K], F8, isOutput=False)
    out_dram = nc.declare_dram_parameter(
        "out", [SEGP * NM, SEGP * C], F32, isOutput=True
    )

    with tile.TileContext(nc) as tc:
        with (
            tc.tile_pool(name="consts", bufs=1) as consts,
            tc.tile_pool(name="xin", bufs=4) as xin,
            tc.tile_pool(name="ohin", bufs=3) as ohin,
            tc.tile_pool(name="big", bufs=2) as big,
            tc.tile_pool(name="stat", bufs=2) as stat,
            tc.tile_pool(name="ps_d", bufs=2, space="PSUM") as psd_pool,
            tc.tile_pool(name="ps_seg", bufs=1, space="PSUM") as psseg,
            tc.tile_pool(name="ps_warm", bufs=1, space="PSUM") as pswarm,
        ):
            cf8_sb = consts.tile([128, 2 * K], F8)
            nc.sync.dma_start(cf8_sb[:], cf8_dram[:])
            const_sb = consts.tile([128, ncst], F32)
            nc.scalar.dma_start(const_sb[:], const_dram[:])

            # PE pstate warmup: keep the tensor engine continuously busy from
            # right after the tiny cf8 load until real tiles arrive
            warm_ps = pswarm.tile([2 * K, 2 * K], F32)
            for _ in range(NWARM):
                nc.tensor.matmul(
                    warm_ps[:], cf8_sb[:], cf8_sb[:], start=True, stop=True
                )

            psum_seg = psseg.tile([SEGP * NM, SEGP * C], F32)

            offs = np.cumsum([0] + gbs)
            st = [dict() for _ in range(nb)]

            def emit_load_mm(b):
                gb, t0, t1 = gbs[b], offs[b], offs[b + 1]
                xb = xin.tile([128, 2, gb, 128], F8, tag="xb")
                nc.sync.dma_start(xb[:, 0], x_dram[:, 0, t0:t1, :])
                nc.sync.dma_start(xb[:, 1], x_dram[:, 1, t0:t1, :])
                oh = ohin.tile([128, gb, C], BF16, tag="oh")
                nc.scalar.dma_start(oh[:], oh_dram[:, t0:t1, :])
                psd = psd_pool.tile([128, gb, K], F32, tag="psd")
                for c in range(2):
                    for g in range(gb):
                        nc.tensor.matmul(
                            psd[:, g, :], xb[:, c, g, :],
                            cf8_sb[:, c * K:(c + 1) * K],
                            start=(c == 0), stop=(c == 1),
                        )
                st[b]["psd"], st[b]["oh"] = psd, oh

            def emit_a(b):
                """PSUM readout + ACT chain for batch b (runs ahead of batch
                b-1's DVE chain)."""
                gb, t0, t1 = gbs[b], offs[b], offs[b + 1]
                t_d2 = big.tile([128, gb, K], F32, tag="t_d2")
                if unit_norm:
                    d2add = _b0(const_sb[:, 0:K], gb, "outer")
                else:
                    d2add = const_sb[:, t0 * K:t1 * K].rearrange(
                        "p (g k) -> p g k", k=K
                    )
                nc.vector.tensor_tensor(t_d2[:], st[b]["psd"][:], d2add, ALU.add)
                lnt = big.tile([128, gb, K], F32, tag="lnt")
                nc.scalar.activation(lnt[:], t_d2[:], ACTF.Ln)
                vals = big.tile([128, gb, NM], BF16, tag="vals")
                nc.scalar.activation(vals[:, :, 0:K], lnt[:], ACTF.Exp, scale=0.5)
                eu = big.tile([128, gb, K], BF16, tag="eu")
                nc.scalar.activation(eu[:], vals[:, :, 0:K], ACTF.Exp, scale=-1.0)
                st[b]["vals"], st[b]["eu"] = vals, eu

            def emit_b(b):
                """Main DVE chain + packed segment matmuls for batch b."""
                gb = gbs[b]
                vals, eu, oh = st[b]["vals"], st[b]["eu"], st[b]["oh"]
                dist = vals[:, :, 0:K]

                m1 = stat.tile([128, gb], F32, tag="m1")
                nc.vector.tensor_reduce(m1[:], dist, axis=AX.X, op=ALU.min)
                mask = big.tile([128, gb, K], BF16, tag="mask")
                nc.vector.tensor_tensor(
                    mask[:], dist, _b0(m1[:], K), ALU.is_equal
                )
                dmask = big.tile([128, gb, K], BF16, tag="dmask")
                nc.vector.scalar_tensor_tensor(
                    dmask[:], mask[:], BIG, dist, ALU.mult, ALU.add
                )
                m2 = stat.tile([128, gb], F32, tag="m2")
                nc.vector.tensor_reduce(m2[:], dmask[:], axis=AX.X, op=ALU.min)

                s = stat.tile([128, gb], F32, tag="s")
                nc.vector.tensor_reduce(s[:], eu[:], axis=AX.X, op=ALU.add)
                prod = big.tile([128, gb, K], BF16, tag="prod")
                nc.vector.tensor_tensor(prod[:], eu[:], dist, ALU.mult)
                spd = stat.tile([128, gb], F32, tag="spd")
                nc.vector.tensor_reduce(spd[:], prod[:], axis=AX.X, op=ALU.add)
                rs = stat.tile([128, gb], F32, tag="rs")
                nc.vector.reciprocal(rs[:], s[:])
                wd = stat.tile([128, gb], F32, tag="wd")
                nc.vector.tensor_tensor(wd[:], spd[:], rs[:], ALU.mult)

                wd3 = wd[:].rearrange("p (g o) -> p g o", o=1)
                nc.vector.tensor_tensor(vals[:, :, 11:12], wd3, wd3, ALU.mult)

                # v1 = relu(wd+m-m1) -> vals[5]; v2 = relu(wd+m-m2);
                # vals[6:11] = mask * (v2 - v1)
                t1s = stat.tile([128, gb], F32, tag="t1")
                nc.vector.scalar_tensor_tensor(
                    t1s[:], wd[:], MARGIN, m1[:], ALU.add, ALU.subtract
                )
                t2s = stat.tile([128, gb], F32, tag="t2")
                nc.vector.scalar_tensor_tensor(
                    t2s[:], wd[:], MARGIN, m2[:], ALU.add, ALU.subtract
                )
                t13 = t1s[:].rearrange("p (g o) -> p g o", o=1)
                nc.vector.tensor_scalar(
                    vals[:, :, K:K + 1], t13, 0.0, None, ALU.max
                )
                v2 = stat.tile([128, gb], F32, tag="v2")
                nc.vector.tensor_scalar(v2[:], t2s[:], 0.0, None, ALU.max)
                dvv = stat.tile([128, gb], F32, tag="dvv")
                nc.vector.tensor_tensor(
                    dvv[:], v2[:],
                    vals[:, :, K:K + 1].rearrange("p g o -> p (g o)"),
                    ALU.subtract,
                )
                nc.vector.tensor_tensor(
                    vals[:, :, 6:11], mask[:], _b0(dvv[:], K), ALU.mult
                )

                npk = gb // SEGP
                p_base = offs[b] // SEGP
                for p in range(npk):
                    nc.tensor.matmul(
                        psum_seg[:],
                        vals[:, p * SEGP:(p + 1) * SEGP, :].rearrange(
                            "p g m -> p (g m)"
                        ),
                        oh[:, p * SEGP:(p + 1) * SEGP, :].rearrange(
                            "p g c -> p (g c)"
                        ),
                        start=(p_base + p == 0),
                        stop=(p_base + p == tiles // SEGP - 1),
                    )

            # software pipeline: A(b+1) is emitted before B(b) so the PSUM
            # readout + Ln/Exp of the next batch overlap the current DVE chain
            emit_load_mm(0)
            emit_a(0)
            for b in range(nb):
                if b + 1 < nb:
                    emit_load_mm(b + 1)
                    emit_a(b + 1)
                emit_b(b)

            seg_sb = consts.tile([SEGP * NM, SEGP * C], F32, tag="seg_sb")
            nc.vector.tensor_copy(seg_sb[:], psum_seg[:])
            nc.sync.dma_start(out_dram[:], seg_sb[:])

    nc.compile()
    return nc


def _host_prep(feat, labels, label2, centers, tiles, unit_norm, n_cores=NCORES):
    """Filter w==1 rows, pad + shard + pre-transpose + fp8-cast."""
    import ml_dtypes

    rpc = tiles * 128
    bpad = rpc * n_cores

    feat = np.asarray(feat, dtype=np.float32)
    labels = np.asarray(labels)
    label2 = np.asarray(label2)
    centers = np.asarray(centers, dtype=np.float32)

    idx = np.flatnonzero(label2 == 1)
    nw = idx.size

    xpad = np.zeros((bpad, D), dtype=np.float32)
    xpad[:nw] = feat[idx]
    x_f8 = (xpad * XSCALE).astype(ml_dtypes.float8_e4m3)

    wlab = np.full(bpad, C, dtype=np.int64)
    wlab[:nw] = labels[idx]
    oh_full = (wlab[:, None] == np.arange(C)[None, :]).astype(ml_dtypes.bfloat16)

    ctilT = (centers.T * (-2.0 / XSCALE)).astype(ml_dtypes.float8_e4m3)
    cf8 = np.ascontiguousarray(
        np.concatenate([ctilT[0:128], ctilT[128:256]], axis=1)
    )  # [128, 10]
    cnorm = (centers * centers).sum(axis=1).astype(np.float32)  # [5]

    if not unit_norm:
        norm2 = np.einsum("ij,ij->i", xpad, xpad, dtype=np.float32)

    in_maps = []
    for i in range(n_cores):
        sl = slice(i * rpc, (i + 1) * rpc)
        xi = np.ascontiguousarray(
            x_f8[sl].reshape(tiles, 128, 2, 128).transpose(3, 2, 0, 1)
        )
        if unit_norm:
            d2add = np.ascontiguousarray(
                np.tile((cnorm + 1.0)[None, :], (128, 1))
            )
        else:
            n2 = norm2[sl].reshape(tiles, 128).T  # [128, tiles]
            d2add = np.ascontiguousarray(
                (n2[:, :, None] + cnorm[None, None, :]).reshape(128, tiles * K)
            )
        ohi = np.ascontiguousarray(
            oh_full[sl].reshape(tiles, 128, C).transpose(1, 0, 2)
        )
        in_maps.append({"x": xi, "const": d2add, "oh": ohi, "cf8": cf8})
    return in_maps


def _host_final(parts, labels, label2, num_classes):
    """Sum per-core [84,112] partials, extract diagonal [12,16] blocks, and
    do the per-class argmin + final reduction (mirrors the reference)."""
    seg = np.zeros((NM, C), dtype=np.float64)
    if parts:
        S = np.zeros((SEGP * NM, SEGP * C), dtype=np.float64)
        for p in parts:
            S += np.asarray(p, dtype=np.float64)
        for p in range(SEGP):
            seg += S[p * NM:(p + 1) * NM, p * C:(p + 1) * C]

    labels = np.asarray(labels).astype(np.int64)
    label2 = np.asarray(label2)
    Ci = int(num_classes)
    w = (label2 == 1)
    cnt = np.bincount(labels[w], minlength=Ci).astype(np.float64)[:C]
    present = np.bincount(labels, minlength=Ci)[:C] > 0

    safe = np.maximum(cnt, 1.0)
    meand = seg[0:K] / safe[None, :]          # [K, C]
    closest = np.argmin(meand, axis=0)        # [C]
    # sum_c viol = sum v1 + [sum mask_j*(v2-v1)] at the class's closest j
    sv = seg[K] + seg[K + 1 + closest, np.arange(C)]
    has = (cnt > 0).astype(np.float64)
    per_class = (seg[11] + sv) / safe * has
    n_unique = max(float(present.sum()), 1.0)
    return np.float32(per_class.sum() / n_unique)


_NC_CACHE = {}


def kernel(feat_normed, labels, label2, num_classes, centers, _trace=False):
    label2 = np.asarray(label2)
    nw = int((label2 == 1).sum())
    if nw == 0:
        return np.asarray(
            _host_final([], labels, label2, num_classes), dtype=np.float32
        )
    feat_normed = np.asarray(feat_normed, dtype=np.float32)
    # unit-norm fast path (feat_normed is normalized by construction);
    # sampled check with a per-row-|x|^2-table fallback build
    samp = feat_normed[:: max(1, feat_normed.shape[0] // 512)]
    unit_norm = bool(
        np.allclose(np.einsum("ij,ij->i", samp, samp), 1.0, atol=1e-3)
    )
    tiles = -(-nw // (128 * NCORES))          # ceil rows / (128*cores)
    tiles = SEGP * (-(-tiles // SEGP))        # round up to multiple of SEGP
    key = (tiles, unit_norm)
    if key not in _NC_CACHE:
        _NC_CACHE[key] = build_nc(tiles, unit_norm)
    nc = _NC_CACHE[key]
    in_maps = _host_prep(
        feat_normed, labels, label2, centers, tiles, unit_norm
    )
    res = run_bass_kernel_spmd(
        nc, in_maps, core_ids=list(range(NCORES)), trace=_trace
    )
    parts = [r["out"] for r in res.results]
    out = _host_final(parts, labels, label2, num_classes)
    if _trace:
        kernel.last_result = res
    return np.asarray(out, dtype=np.float32)
